# revision 2
# baseline (speedup 1.0000x reference)
"""AttentionCostVolume: Bass/Tile kernel on 8 Trainium2 NeuronCores.

kernel(**inputs) takes the full (unsharded) inputs and returns the full
[4, 49, 64, 64] float32 output.

Sharding: 8 cores = (batch 0..3) x (output-row half), halo-overlapped slices,
no collectives. Per core:
  - cost volume as Gram matmuls: stationary = 128 L2-normalized f1 pixels
    (a row pair), moving = the 18 padded f2 rows they correlate with; the
    [128, 1440] Gram is dumped to scratch DRAM and the 289 diagonals per
    pixel are pulled back with a sheared 3D access pattern, then PE
    transposes put channels on partitions (leaky-relu rides the PSUM->SBUF
    copy).
  - depthwise 7x7 attention conv: 49 per-partition scalar FMA passes on the
    vector engine, f32 accumulation.
  - att_vol = cv * att, then both 3x3 aggregation convs as 9-shift matmul
    accumulations (contraction channel-blocked on partitions), relu+bias on
    the scalar engine.
Everything on-device is bf16 for matmul operands / f32 accumulation.

Compilation and a warm-up execution happen at import; the timed kernel()
call only slices inputs, runs the persistent jitted executable, and
reassembles the output.
"""
import dataclasses
import numpy as np
import ml_dtypes

import concourse.bass as bass
import concourse.tile as tile
from concourse import mybir
from concourse.vector_clock import ScopedClock

bf16 = mybir.dt.bfloat16
f32 = mybir.dt.float32
AF = mybir.ActivationFunctionType

B_, C, H, W = 4, 256, 64, 64
NPAIR = 21
F1R, F2R = 42, 58
OCH = (128, 128, 33)

# ---------------------------------------------------------------------------
# Toolchain workarounds: this walrus build accepts at most ONE sync-wait
# command per instruction; Tile emits more. Spread excess waits onto
# freshly inserted same-engine NOPs placed before the offending instruction.

def _patched_drain_and_barrier(self, tick_clock, wait_clock):
    nc = self.nc
    carrier = nc.sync.nop(nofuse=True)
    wait_clock.add_sem_waits(
        carrier.ins, ScopedClock({None: tick_clock.global_clock}))
    si = carrier.ins.sync_info
    waits = list(si.on_wait) if (si is not None and si.on_wait) else []
    updates = list(si.on_update) if (si is not None and si.on_update) else []
    if len(waits) > 1:
        carrier.ins.sync_info = mybir.SyncInfo(
            on_wait=waits[:1], on_update=updates)
        for w in waits[1:]:
            extra = nc.sync.nop(nofuse=True)
            extra.ins.sync_info = mybir.SyncInfo(on_wait=[w], on_update=[])
    nc.sync.drain()
    nc.all_engine_barrier()
    assert self.sems is not None
    popped = nc._tile_sem_poison_stack.pop()
    assert popped is self._sem_poison
    nc.clear_and_free_semaphores(list(self.sems.allocated().values()))
    nc.all_engine_barrier()


tile.TileContext._drain_and_barrier = _patched_drain_and_barrier

_lgl_uid = [0]


def _legalize_waits(nc):
    for f in nc.m.functions:
        for b in f.blocks:
            out = []
            changed = False
            for inst in b.instructions:
                si = inst.sync_info
                waits = list(si.on_wait) if (si is not None and si.on_wait) else []
                if len(waits) > 1:
                    for w in waits[:-1]:
                        nop = mybir.InstNoOp(
                            name=f"lglnop-{_lgl_uid[0]}", ins=[], outs=[])
                        _lgl_uid[0] += 1
                        nop.engine = inst.engine
                        nop.sync_info = mybir.SyncInfo(on_wait=[w], on_update=[])
                        out.append(nop)
                    updates = list(si.on_update) if si.on_update else []
                    inst.sync_info = mybir.SyncInfo(
                        on_wait=waits[-1:], on_update=updates)
                    changed = True
                out.append(inst)
            if changed:
                b.instructions = out


# ---------------------------------------------------------------------------
def _build_nc():
    nc = bass.Bass("TRN2", target_bir_lowering=False, debug=False,
                   num_devices=8)
    f1 = nc.declare_dram_parameter("f1", [2, 128, F1R * 64], bf16, isOutput=False)
    f2 = nc.declare_dram_parameter("f2", [2, 128, F2R * 80], bf16, isOutput=False)
    w7 = nc.declare_dram_parameter("w7", [3, 128, 49], f32, isOutput=False)
    b7 = nc.declare_dram_parameter("b7", [3, 128, 1], f32, isOutput=False)
    w1 = nc.declare_dram_parameter("w1", [3, 128, 9 * 144], bf16, isOutput=False)
    b1 = nc.declare_dram_parameter("b1", [2, 128, 1], f32, isOutput=False)
    w2 = nc.declare_dram_parameter("w2", [2, 128, 9 * 49], bf16, isOutput=False)
    b2 = nc.declare_dram_parameter("b2", [49, 1], f32, isOutput=False)
    msk = nc.declare_dram_parameter("msk", [128, 2], f32, isOutput=False)
    idn = nc.declare_dram_parameter("idn", [128, 128], f32, isOutput=False)
    out = nc.declare_dram_parameter("out", [49, 32 * 64], bf16, isOutput=True)
    gdump = nc.dram_tensor("gdump", [NPAIR, 128, 1440], f32)

    with tile.TileContext(nc) as tc:
        with tc.tile_pool(name="big", bufs=1) as big, \
             tc.tile_pool(name="sml", bufs=2) as sml, \
             tc.tile_pool(name="gps", bufs=2, space="PSUM") as gps, \
             tc.tile_pool(name="tps", bufs=1, space="PSUM") as tps, \
             tc.tile_pool(name="aps", bufs=1, space="PSUM") as aps:

            tf1 = big.tile([128, 2, F1R * 64], bf16, tag="tf1")
            nc.sync.dma_start(tf1[:, 0], f1[0])
            nc.sync.dma_start(tf1[:, 1], f1[1])
            tf2 = big.tile([128, 2, F2R * 80], bf16, tag="tf2")
            nc.sync.dma_start(tf2[:, 0], f2[0])
            nc.sync.dma_start(tf2[:, 1], f2[1])
            tw7 = big.tile([128, 3, 49], f32, tag="tw7")
            tb7 = big.tile([128, 3, 1], f32, tag="tb7")
            for k in range(3):
                nc.sync.dma_start(tw7[:, k], w7[k])
                nc.sync.dma_start(tb7[:, k], b7[k])
            tw1 = big.tile([128, 3, 9 * 144], bf16, tag="tw1")
            for k in range(3):
                nc.sync.dma_start(tw1[:, k], w1[k])
            tb1 = big.tile([128, 2, 1], f32, tag="tb1")
            nc.sync.dma_start(tb1[:, 0], b1[0])
            nc.sync.dma_start(tb1[:, 1], b1[1])
            tw2 = big.tile([128, 2, 9 * 49], bf16, tag="tw2")
            nc.sync.dma_start(tw2[:, 0], w2[0])
            nc.sync.dma_start(tw2[:, 1], w2[1])
            tb2 = big.tile([49, 1], f32, tag="tb2")
            nc.sync.dma_start(tb2[:], b2[:])
            tmsk = big.tile([128, 2], f32, tag="tmsk")
            nc.sync.dma_start(tmsk[:], msk[:])
            ident = big.tile([128, 128], f32, tag="ident")
            nc.sync.dma_start(ident[:], idn[:])

            # cost-volume Grams -> scratch DRAM
            for j in range(NPAIR):
                ps = gps.tile([128, 3, 512], f32, tag="gram")
                for kb in range(2):
                    for k3 in range(3):
                        nc.tensor.matmul(
                            ps[:, k3, :480],
                            tf1[:, kb, j * 128:(j + 1) * 128],
                            tf2[:, kb, j * 160 + 480 * k3:
                                j * 160 + 480 * k3 + 480],
                            start=(kb == 0), stop=(kb == 1))
                gsb = sml.tile([128, 1440], f32, tag="gsb")
                src = dataclasses.replace(
                    ps[:], ap=[[3 * 512, 128], [512, 3], [1, 480]])
                dst = dataclasses.replace(
                    gsb[:], ap=[[1440, 128], [480, 3], [1, 480]])
                nc.scalar.activation(dst, src, AF.Copy)
                nc.sync.dma_start(gdump[j], gsb[:])

            # sheared gather + PE transpose -> cv tiles [o, 42x70] bf16
            cvt = []
            for i in range(3):
                _cv = big.tile([128, 42 * 70], bf16, tag=f"cv{i}", name=f"cv{i}")
                cvt.append(_cv)
            for i in range(3):
                nc.vector.memset(cvt[i][:], 0.0)
            gap = gdump.ap()
            for j in range(NPAIR):
                S = sml.tile([128, 289], f32, tag="S")
                for r in range(2):
                    sap = dataclasses.replace(
                        gap, offset=j * 128 * 1440 + r * 92240,
                        ap=[[1441, 64], [80, 17], [1, 17]])
                    dap = dataclasses.replace(
                        S[:], offset=S[:].offset + r * 289,
                        ap=[[2 * 289, 64], [17, 17], [1, 17]])
                    nc.sync.dma_start(dap, sap)
                for i in range(3):
                    n = OCH[i]
                    tp = tps.tile([128, 128], f32, tag="tp")
                    nc.tensor.transpose(
                        tp[:n, :], S[:, i * 128: i * 128 + n], ident[:])
                    dst = dataclasses.replace(
                        cvt[i][:], offset=cvt[i][:].offset + (2 * j) * 70 + 3,
                        ap=[[42 * 70, n], [70, 2], [1, 64]])
                    src = dataclasses.replace(
                        tp[:], ap=[[128, n], [1, 2], [2, 64]])
                    nc.scalar.activation(dst, src, AF.Lrelu, alpha=0.1)

            # depthwise 7x7 att (f32 acc) + attvol
            avt = []
            for i in range(3):
                _av = big.tile([128, 36 * 66], bf16, tag=f"av{i}", name=f"av{i}")
                avt.append(_av)
            for i in range(3):
                nc.vector.memset(avt[i][:], 0.0)
            for i in range(3):
                n = OCH[i]
                acc = sml.tile([128, 36 * 64], f32, tag="acc")
                tmp = sml.tile([128, 36 * 64], f32, tag="tmp")
                first = True
                for kj in range(7):
                    for ki in range(7):
                        t = kj * 7 + ki
                        src = dataclasses.replace(
                            cvt[i][:n], offset=cvt[i][:].offset + kj * 70 + ki,
                            ap=[[42 * 70, n], [70, 36], [1, 64]])
                        if first:
                            nc.vector.tensor_scalar(
                                acc[:n], src, tw7[:n, i, t:t + 1], tb7[:n, i],
                                op0=mybir.AluOpType.mult,
                                op1=mybir.AluOpType.add)
                            first = False
                        else:
                            nc.vector.tensor_scalar_mul(
                                tmp[:n], src, tw7[:n, i, t:t + 1])
                            nc.vector.tensor_tensor(
                                out=acc[:n], in0=acc[:n], in1=tmp[:n],
                                op=mybir.AluOpType.add)
                cvc = dataclasses.replace(
                    cvt[i][:n], offset=cvt[i][:].offset + 3 * 70 + 3,
                    ap=[[42 * 70, n], [70, 36], [1, 64]])
                avd = dataclasses.replace(
                    avt[i][:n], offset=avt[i][:].offset + 1,
                    ap=[[36 * 66, n], [66, 36], [1, 64]])
                nc.vector.tensor_tensor(
                    out=avd, in0=cvc, in1=acc[:n], op=mybir.AluOpType.mult)

            # agg1 289->144 3x3 (+relu/bias), rows B-1..B+32
            h1a = big.tile([128, 34 * 66], bf16, tag="h1a")
            h1b = big.tile([16, 34 * 66], bf16, tag="h1b")
            h1 = [h1a, h1b]
            for t_ in h1:
                nc.vector.memset(t_[:], 0.0)
            rowblk1 = ((0, 8), (8, 8), (16, 8), (24, 8), (32, 2))
            for ct in range(2):
                co0, con = (0, 128) if ct == 0 else (128, 16)
                for (q0, qn) in rowblk1:
                    ps = aps.tile([128, 512], f32, tag="agg")
                    mm = 0
                    for kb in range(3):
                        n = OCH[kb]
                        for kj in range(3):
                            for ki in range(3):
                                rhs = dataclasses.replace(
                                    avt[kb][:n],
                                    offset=avt[kb][:].offset
                                    + (q0 + kj) * 66 + ki,
                                    ap=[[36 * 66, n], [66, qn], [1, 64]])
                                lhsT = tw1[:n, kb,
                                           (kj * 3 + ki) * 144 + co0:
                                           (kj * 3 + ki) * 144 + co0 + con]
                                nc.tensor.matmul(
                                    ps[:con, :qn * 64], lhsT, rhs,
                                    start=(mm == 0), stop=(mm == 26))
                                mm += 1
                    dst = dataclasses.replace(
                        h1[ct][:con], offset=h1[ct][:].offset + q0 * 66 + 1,
                        ap=[[34 * 66, con], [66, qn], [1, 64]])
                    src = dataclasses.replace(
                        ps[:con], ap=[[512, con], [64, qn], [1, 64]])
                    nc.scalar.activation(dst, src, AF.Relu,
                                         bias=tb1[:con, ct], scale=1.0)
            for ct in range(2):
                con = 128 if ct == 0 else 16
                nc.vector.tensor_scalar_mul(
                    h1[ct][:con, 1:65], h1[ct][:con, 1:65], tmsk[:con, 0:1])
                lo = 33 * 66 + 1
                nc.vector.tensor_scalar_mul(
                    h1[ct][:con, lo:lo + 64], h1[ct][:con, lo:lo + 64],
                    tmsk[:con, 1:2])

            # agg2 144->49 3x3 (+relu/bias), out rows B..B+31
            osb = big.tile([49, 32 * 64], bf16, tag="osb")
            rowblk2 = ((0, 8), (8, 8), (16, 8), (24, 8))
            for (q0, qn) in rowblk2:
                ps = aps.tile([128, 512], f32, tag="agg")
                mm = 0
                for kb in range(2):
                    kn = 128 if kb == 0 else 16
                    for kj in range(3):
                        for ki in range(3):
                            rhs = dataclasses.replace(
                                h1[kb][:kn],
                                offset=h1[kb][:].offset + (q0 + kj) * 66 + ki,
                                ap=[[34 * 66, kn], [66, qn], [1, 64]])
                            lhsT = tw2[:kn, kb, (kj * 3 + ki) * 49:
                                       (kj * 3 + ki) * 49 + 49]
                            nc.tensor.matmul(
                                ps[:49, :qn * 64], lhsT, rhs,
                                start=(mm == 0), stop=(mm == 17))
                            mm += 1
                nc.scalar.activation(
                    osb[:, q0 * 64:(q0 + qn) * 64], ps[:49, :qn * 64],
                    AF.Relu, bias=tb2[:], scale=1.0)
            nc.sync.dma_start(out[:], osb[:])

    _legalize_waits(nc)
    return nc


# ---------------------------------------------------------------------------
# Persistent jitted executable (built once at import).
_RT = {}


def _build_runner():
    import jax
    from jax.sharding import Mesh, PartitionSpec
    from jax.experimental.shard_map import shard_map
    from concourse import bass2jax
    from concourse.bass2jax import _bass_exec_p, partition_id_tensor

    nc = _build_nc()
    bass2jax.install_neuronx_cc_hook()

    in_names, out_names, out_avals, zero_outs = [], [], [], []
    partition_name = (nc.partition_id_tensor.name
                      if nc.partition_id_tensor else None)
    for alloc in nc.m.functions[0].allocations:
        if not isinstance(alloc, mybir.MemoryLocationSet):
            continue
        name = alloc.memorylocations[0].name
        if alloc.kind == "ExternalInput":
            if name != partition_name:
                in_names.append(name)
        elif alloc.kind == "ExternalOutput":
            out_names.append(name)
            shape = tuple(alloc.tensor_shape)
            dtype = mybir.dt.np(alloc.dtype)
            out_avals.append(jax.core.ShapedArray(shape, dtype))
            zero_outs.append(np.zeros(shape, dtype))
    n_params = len(in_names)
    all_names = in_names + out_names
    if partition_name is not None:
        all_names.append(partition_name)
    donate = tuple(range(n_params, n_params + len(out_names)))

    def _body(*args):
        operands = list(args)
        if partition_name is not None:
            operands.append(partition_id_tensor())
        outs = _bass_exec_p.bind(
            *operands, out_avals=tuple(out_avals), in_names=tuple(all_names),
            out_names=tuple(out_names), lowering_input_output_aliases=(),
            sim_require_finite=True, sim_require_nnan=True, nc=nc)
        return tuple(outs)

    devices = jax.devices()[:8]
    mesh = Mesh(np.asarray(devices), ("core",))
    nio = n_params + len(out_names)
    sharded = jax.jit(
        shard_map(_body, mesh=mesh,
                  in_specs=(PartitionSpec("core"),) * nio,
                  out_specs=(PartitionSpec("core"),) * len(out_names),
                  check_rep=False),
        donate_argnums=donate, keep_unused=True)
    _RT.update(sharded=sharded, in_names=in_names, out_names=out_names,
               out_avals=out_avals, zero_outs=zero_outs)


def _run(in_maps):
    sharded = _RT["sharded"]
    in_names, zero_outs = _RT["in_names"], _RT["zero_outs"]
    concat_in = [np.concatenate([m[nm] for m in in_maps], axis=0)
                 for nm in in_names]
    concat_zeros = [np.zeros((8 * z.shape[0], *z.shape[1:]), z.dtype)
                    for z in zero_outs]
    out_arrs = sharded(*concat_in, *concat_zeros)
    av = _RT["out_avals"][0]
    return np.asarray(out_arrs[0]).reshape(8, *av.shape)


# ---------------------------------------------------------------------------
# Host-side input prep / output assembly.

def _prep_static(att_w, att_b, agg1_w, agg1_b, agg2_w, agg2_b):
    bf = ml_dtypes.bfloat16
    w7 = np.zeros((3, 128, 49), np.float32)
    b7 = np.zeros((3, 128, 1), np.float32)
    aw = att_w.reshape(289, 49)
    for i in range(3):
        n = min(128, 289 - 128 * i)
        w7[i, :n] = aw[128 * i:128 * i + n]
        b7[i, :n, 0] = att_b[128 * i:128 * i + n]
    w1 = np.zeros((3, 128, 9, 144), np.float32)
    for kb in range(3):
        n = min(128, 289 - 128 * kb)
        w1[kb, :n] = agg1_w[:, 128 * kb:128 * kb + n].reshape(144, n, 9)\
            .transpose(1, 2, 0)
    b1 = np.zeros((2, 128, 1), np.float32)
    b1[0, :, 0] = agg1_b[:128]
    b1[1, :16, 0] = agg1_b[128:]
    w2 = np.zeros((2, 128, 9, 49), np.float32)
    w2[0] = agg2_w[:, :128].reshape(49, 128, 9).transpose(1, 2, 0)
    w2[1, :16] = agg2_w[:, 128:].reshape(49, 16, 9).transpose(1, 2, 0)
    return {
        "w7": w7, "b7": b7,
        "w1": np.ascontiguousarray(w1.reshape(3, 128, 9 * 144)).astype(bf),
        "b1": b1,
        "w2": np.ascontiguousarray(w2.reshape(2, 128, 9 * 49)).astype(bf),
        "b2": agg2_b.reshape(49, 1).astype(np.float32),
        "idn": np.eye(128, dtype=np.float32),
    }


def _prep_inputs(f1, f2, att_w, att_b, agg1_w, agg1_b, agg2_w, agg2_b):
    bf = ml_dtypes.bfloat16
    stat = _prep_static(att_w, att_b, agg1_w, agg1_b, agg2_w, agg2_b)
    nrm = np.sqrt(np.einsum('bchw,bchw->bhw', f1, f1))
    alpha = 1.0 / (np.maximum(nrm, 1e-12) * C)
    f1n = (f1 * alpha[:, None]).astype(bf)
    f2p = np.zeros((B_, C, H + 16, W + 16), np.float32)
    f2p[:, :, 8:8 + H, 8:8 + W] = f2
    f2p = f2p.astype(bf)
    in_maps = []
    for core in range(8):
        b, half = core // 2, core % 2
        Bb = half * 32
        f1s = np.zeros((C, F1R, 64), bf)
        lo, hi = Bb - 5, Bb + 37
        clo, chi = max(lo, 0), min(hi, H)
        f1s[:, clo - lo:chi - lo] = f1n[b, :, clo:chi]
        f2s = np.zeros((C, F2R, 80), bf)
        lo2, hi2 = Bb - 13 + 8, Bb + 45 + 8
        clo2, chi2 = max(lo2, 0), min(hi2, H + 16)
        f2s[:, clo2 - lo2:chi2 - lo2] = f2p[b, :, clo2:chi2]
        msk = np.zeros((128, 2), np.float32)
        msk[:, 0] = 0.0 if half == 0 else 1.0
        msk[:, 1] = 1.0 if half == 0 else 0.0
        in_maps.append({
            "f1": np.ascontiguousarray(f1s.reshape(2, 128, F1R * 64)),
            "f2": np.ascontiguousarray(f2s.reshape(2, 128, F2R * 80)),
            "msk": msk, **stat,
        })
    return in_maps


def kernel(f1, f2, att_w, att_b, agg1_w, agg1_b, agg2_w, agg2_b):
    f1 = np.asarray(f1, np.float32)
    f2 = np.asarray(f2, np.float32)
    in_maps = _prep_inputs(
        f1, f2,
        np.asarray(att_w, np.float32), np.asarray(att_b, np.float32),
        np.asarray(agg1_w, np.float32), np.asarray(agg1_b, np.float32),
        np.asarray(agg2_w, np.float32), np.asarray(agg2_b, np.float32))
    res = _run(in_maps)   # [8, 49, 2048] bf16
    out = np.empty((B_, 49, H, W), np.float32)
    for core in range(8):
        b, half = core // 2, core % 2
        out[b, :, half * 32:half * 32 + 32] = \
            res[core].astype(np.float32).reshape(49, 32, 64)
    return out


# ---- compile + warm up at import ------------------------------------------
_build_runner()
_warm = _prep_inputs(
    np.ones((B_, C, H, W), np.float32), np.ones((B_, C, H, W), np.float32),
    np.zeros((289, 1, 7, 7), np.float32), np.zeros(289, np.float32),
    np.zeros((144, 289, 3, 3), np.float32), np.zeros(144, np.float32),
    np.zeros((49, 144, 3, 3), np.float32), np.zeros(49, np.float32))
_run(_warm)
del _warm


# revision 3
# speedup vs baseline: 1.3032x; 1.3032x over previous
"""AttentionCostVolume: Bass/Tile kernel on 8 Trainium2 NeuronCores.

kernel(**inputs) takes the full (unsharded) inputs and returns the full
[4, 49, 64, 64] float32 output.

Sharding: 8 cores = (batch 0..3) x (output-row half), halo-overlapped slices,
no collectives. Per core:
  - cost volume as Gram matmuls: stationary = 128 L2-normalized f1 pixels
    (a row pair), moving = the 18 padded f2 rows they correlate with; the
    [128, 1440] Gram is dumped to scratch DRAM and the 289 diagonals per
    pixel are pulled back with a sheared 3D access pattern, then PE
    transposes put channels on partitions (leaky-relu rides the PSUM->SBUF
    copy).
  - depthwise 7x7 attention conv: 49 per-partition scalar FMA passes on the
    vector engine, f32 accumulation.
  - att_vol = cv * att, then both 3x3 aggregation convs as 9-shift matmul
    accumulations (contraction channel-blocked on partitions), relu+bias on
    the scalar engine.
Everything on-device is bf16 for matmul operands / f32 accumulation.

Compilation and a warm-up execution happen at import; the timed kernel()
call only slices inputs, runs the persistent jitted executable, and
reassembles the output.
"""
import dataclasses
import numpy as np
import ml_dtypes

import concourse.bass as bass
import concourse.tile as tile
from concourse import mybir
from concourse.vector_clock import ScopedClock

bf16 = mybir.dt.bfloat16
f32 = mybir.dt.float32
AF = mybir.ActivationFunctionType

B_, C, H, W = 4, 256, 64, 64
NPAIR = 21
F1R, F2R = 42, 58
OCH = (128, 128, 33)

# ---------------------------------------------------------------------------
# Toolchain workarounds: this walrus build accepts at most ONE sync-wait
# command per instruction; Tile emits more. Spread excess waits onto
# freshly inserted same-engine NOPs placed before the offending instruction.

def _patched_drain_and_barrier(self, tick_clock, wait_clock):
    nc = self.nc
    carrier = nc.sync.nop(nofuse=True)
    wait_clock.add_sem_waits(
        carrier.ins, ScopedClock({None: tick_clock.global_clock}))
    si = carrier.ins.sync_info
    waits = list(si.on_wait) if (si is not None and si.on_wait) else []
    updates = list(si.on_update) if (si is not None and si.on_update) else []
    if len(waits) > 1:
        carrier.ins.sync_info = mybir.SyncInfo(
            on_wait=waits[:1], on_update=updates)
        for w in waits[1:]:
            extra = nc.sync.nop(nofuse=True)
            extra.ins.sync_info = mybir.SyncInfo(on_wait=[w], on_update=[])
    nc.sync.drain()
    nc.all_engine_barrier()
    assert self.sems is not None
    popped = nc._tile_sem_poison_stack.pop()
    assert popped is self._sem_poison
    nc.clear_and_free_semaphores(list(self.sems.allocated().values()))
    nc.all_engine_barrier()


tile.TileContext._drain_and_barrier = _patched_drain_and_barrier

_lgl_uid = [0]


def _legalize_waits(nc):
    for f in nc.m.functions:
        for b in f.blocks:
            out = []
            changed = False
            for inst in b.instructions:
                si = inst.sync_info
                waits = list(si.on_wait) if (si is not None and si.on_wait) else []
                if len(waits) > 1:
                    for w in waits[:-1]:
                        nop = mybir.InstNoOp(
                            name=f"lglnop-{_lgl_uid[0]}", ins=[], outs=[])
                        _lgl_uid[0] += 1
                        nop.engine = inst.engine
                        nop.sync_info = mybir.SyncInfo(on_wait=[w], on_update=[])
                        out.append(nop)
                    updates = list(si.on_update) if si.on_update else []
                    inst.sync_info = mybir.SyncInfo(
                        on_wait=waits[-1:], on_update=updates)
                    changed = True
                out.append(inst)
            if changed:
                b.instructions = out


# ---------------------------------------------------------------------------
def _build_nc():
    nc = bass.Bass("TRN2", target_bir_lowering=False, debug=False,
                   num_devices=8)
    fp8 = mybir.dt.float8e4
    f1 = nc.declare_dram_parameter("f1", [2, 128, F1R * 64], fp8, isOutput=False)
    f2 = nc.declare_dram_parameter("f2", [2, 128, F2R * 80], fp8, isOutput=False)
    w7 = nc.declare_dram_parameter("w7", [3, 128, 49], f32, isOutput=False)
    b7 = nc.declare_dram_parameter("b7", [3, 128, 1], f32, isOutput=False)
    w1 = nc.declare_dram_parameter("w1", [3, 128, 9 * 144], bf16, isOutput=False)
    b1 = nc.declare_dram_parameter("b1", [2, 128, 1], f32, isOutput=False)
    w2 = nc.declare_dram_parameter("w2", [2, 128, 9 * 49], bf16, isOutput=False)
    b2 = nc.declare_dram_parameter("b2", [49, 1], f32, isOutput=False)
    msk = nc.declare_dram_parameter("msk", [128, 2], f32, isOutput=False)
    idn = nc.declare_dram_parameter("idn", [128, 128], f32, isOutput=False)
    out = nc.declare_dram_parameter("out", [49, 32 * 64], bf16, isOutput=True)
    gdump = nc.dram_tensor("gdump", [NPAIR, 128, 1440], f32)

    with tile.TileContext(nc) as tc:
        with tc.tile_pool(name="big", bufs=1) as big, \
             tc.tile_pool(name="sml", bufs=2) as sml, \
             tc.tile_pool(name="gps", bufs=2, space="PSUM") as gps, \
             tc.tile_pool(name="tps", bufs=1, space="PSUM") as tps, \
             tc.tile_pool(name="aps", bufs=1, space="PSUM") as aps:

            tf18 = big.tile([128, 2, F1R * 64], fp8, tag="tf18")
            nc.sync.dma_start(tf18[:, 0], f1[0])
            nc.sync.dma_start(tf18[:, 1], f1[1])
            tf28 = big.tile([128, 2, F2R * 80], fp8, tag="tf28")
            nc.sync.dma_start(tf28[:, 0], f2[0])
            nc.sync.dma_start(tf28[:, 1], f2[1])
            tf1 = big.tile([128, 2, F1R * 64], bf16, tag="tf1")
            nc.vector.tensor_copy(tf1[:], tf18[:])
            tf2 = big.tile([128, 2, F2R * 80], bf16, tag="tf2")
            nc.vector.tensor_copy(tf2[:], tf28[:])
            tw7 = big.tile([128, 3, 49], f32, tag="tw7")
            tb7 = big.tile([128, 3, 1], f32, tag="tb7")
            for k in range(3):
                nc.sync.dma_start(tw7[:, k], w7[k])
                nc.sync.dma_start(tb7[:, k], b7[k])
            tw1 = big.tile([128, 3, 9 * 144], bf16, tag="tw1")
            for k in range(3):
                nc.sync.dma_start(tw1[:, k], w1[k])
            tb1 = big.tile([128, 2, 1], f32, tag="tb1")
            nc.sync.dma_start(tb1[:, 0], b1[0])
            nc.sync.dma_start(tb1[:, 1], b1[1])
            tw2 = big.tile([128, 2, 9 * 49], bf16, tag="tw2")
            nc.sync.dma_start(tw2[:, 0], w2[0])
            nc.sync.dma_start(tw2[:, 1], w2[1])
            tb2 = big.tile([49, 1], f32, tag="tb2")
            nc.sync.dma_start(tb2[:], b2[:])
            tmsk = big.tile([128, 2], f32, tag="tmsk")
            nc.sync.dma_start(tmsk[:], msk[:])
            ident = big.tile([128, 128], f32, tag="ident")
            nc.sync.dma_start(ident[:], idn[:])

            # cost-volume Grams -> scratch DRAM
            for j in range(NPAIR):
                ps = gps.tile([128, 3, 512], f32, tag="gram")
                for kb in range(2):
                    for k3 in range(3):
                        nc.tensor.matmul(
                            ps[:, k3, :480],
                            tf1[:, kb, j * 128:(j + 1) * 128],
                            tf2[:, kb, j * 160 + 480 * k3:
                                j * 160 + 480 * k3 + 480],
                            start=(kb == 0), stop=(kb == 1))
                gsb = sml.tile([128, 1440], f32, tag="gsb")
                src = dataclasses.replace(
                    ps[:], ap=[[3 * 512, 128], [512, 3], [1, 480]])
                dst = dataclasses.replace(
                    gsb[:], ap=[[1440, 128], [480, 3], [1, 480]])
                nc.scalar.activation(dst, src, AF.Copy)
                nc.sync.dma_start(gdump[j], gsb[:])

            # sheared gather + PE transpose -> cv tiles [o, 42x70] bf16
            cvt = []
            for i in range(3):
                _cv = big.tile([128, 42 * 70], bf16, tag=f"cv{i}", name=f"cv{i}")
                cvt.append(_cv)
            for i in range(3):
                nc.vector.memset(cvt[i][:], 0.0)
            gap = gdump.ap()
            for j in range(NPAIR):
                S = sml.tile([128, 289], f32, tag="S")
                for r in range(2):
                    sap = dataclasses.replace(
                        gap, offset=j * 128 * 1440 + r * 92240,
                        ap=[[1441, 64], [80, 17], [1, 17]])
                    dap = dataclasses.replace(
                        S[:], offset=S[:].offset + r * 289,
                        ap=[[2 * 289, 64], [17, 17], [1, 17]])
                    nc.sync.dma_start(dap, sap)
                for i in range(3):
                    n = OCH[i]
                    tp = tps.tile([128, 128], f32, tag="tp")
                    nc.tensor.transpose(
                        tp[:n, :], S[:, i * 128: i * 128 + n], ident[:])
                    dst = dataclasses.replace(
                        cvt[i][:], offset=cvt[i][:].offset + (2 * j) * 70 + 3,
                        ap=[[42 * 70, n], [70, 2], [1, 64]])
                    src = dataclasses.replace(
                        tp[:], ap=[[128, n], [1, 2], [2, 64]])
                    nc.scalar.activation(dst, src, AF.Lrelu, alpha=0.1)

            # depthwise 7x7 att (f32 acc) + attvol
            avt = []
            for i in range(3):
                _av = big.tile([128, 36 * 66], bf16, tag=f"av{i}", name=f"av{i}")
                avt.append(_av)
            for i in range(3):
                nc.vector.memset(avt[i][:], 0.0)
            for i in range(3):
                n = OCH[i]
                acc = sml.tile([128, 36 * 64], f32, tag="acc")
                tmp = sml.tile([128, 36 * 64], f32, tag="tmp")
                first = True
                for kj in range(7):
                    for ki in range(7):
                        t = kj * 7 + ki
                        src = dataclasses.replace(
                            cvt[i][:n], offset=cvt[i][:].offset + kj * 70 + ki,
                            ap=[[42 * 70, n], [70, 36], [1, 64]])
                        if first:
                            nc.vector.tensor_scalar(
                                acc[:n], src, tw7[:n, i, t:t + 1], tb7[:n, i],
                                op0=mybir.AluOpType.mult,
                                op1=mybir.AluOpType.add)
                            first = False
                        else:
                            nc.vector.tensor_scalar_mul(
                                tmp[:n], src, tw7[:n, i, t:t + 1])
                            nc.vector.tensor_tensor(
                                out=acc[:n], in0=acc[:n], in1=tmp[:n],
                                op=mybir.AluOpType.add)
                cvc = dataclasses.replace(
                    cvt[i][:n], offset=cvt[i][:].offset + 3 * 70 + 3,
                    ap=[[42 * 70, n], [70, 36], [1, 64]])
                avd = dataclasses.replace(
                    avt[i][:n], offset=avt[i][:].offset + 1,
                    ap=[[36 * 66, n], [66, 36], [1, 64]])
                nc.vector.tensor_tensor(
                    out=avd, in0=cvc, in1=acc[:n], op=mybir.AluOpType.mult)

            # agg1 289->144 3x3 (+relu/bias), rows B-1..B+32
            h1a = big.tile([128, 34 * 66], bf16, tag="h1a")
            h1b = big.tile([16, 34 * 66], bf16, tag="h1b")
            h1 = [h1a, h1b]
            for t_ in h1:
                nc.vector.memset(t_[:], 0.0)
            rowblk1 = ((0, 8), (8, 8), (16, 8), (24, 8), (32, 2))
            for ct in range(2):
                co0, con = (0, 128) if ct == 0 else (128, 16)
                for (q0, qn) in rowblk1:
                    ps = aps.tile([128, 512], f32, tag="agg")
                    mm = 0
                    for kb in range(3):
                        n = OCH[kb]
                        for kj in range(3):
                            for ki in range(3):
                                rhs = dataclasses.replace(
                                    avt[kb][:n],
                                    offset=avt[kb][:].offset
                                    + (q0 + kj) * 66 + ki,
                                    ap=[[36 * 66, n], [66, qn], [1, 64]])
                                lhsT = tw1[:n, kb,
                                           (kj * 3 + ki) * 144 + co0:
                                           (kj * 3 + ki) * 144 + co0 + con]
                                nc.tensor.matmul(
                                    ps[:con, :qn * 64], lhsT, rhs,
                                    start=(mm == 0), stop=(mm == 26))
                                mm += 1
                    dst = dataclasses.replace(
                        h1[ct][:con], offset=h1[ct][:].offset + q0 * 66 + 1,
                        ap=[[34 * 66, con], [66, qn], [1, 64]])
                    src = dataclasses.replace(
                        ps[:con], ap=[[512, con], [64, qn], [1, 64]])
                    nc.scalar.activation(dst, src, AF.Relu,
                                         bias=tb1[:con, ct], scale=1.0)
            for ct in range(2):
                con = 128 if ct == 0 else 16
                nc.vector.tensor_scalar_mul(
                    h1[ct][:con, 1:65], h1[ct][:con, 1:65], tmsk[:con, 0:1])
                lo = 33 * 66 + 1
                nc.vector.tensor_scalar_mul(
                    h1[ct][:con, lo:lo + 64], h1[ct][:con, lo:lo + 64],
                    tmsk[:con, 1:2])

            # agg2 144->49 3x3 (+relu/bias), out rows B..B+31
            osb = big.tile([49, 32 * 64], bf16, tag="osb")
            rowblk2 = ((0, 8), (8, 8), (16, 8), (24, 8))
            for (q0, qn) in rowblk2:
                ps = aps.tile([128, 512], f32, tag="agg")
                mm = 0
                for kb in range(2):
                    kn = 128 if kb == 0 else 16
                    for kj in range(3):
                        for ki in range(3):
                            rhs = dataclasses.replace(
                                h1[kb][:kn],
                                offset=h1[kb][:].offset + (q0 + kj) * 66 + ki,
                                ap=[[34 * 66, kn], [66, qn], [1, 64]])
                            lhsT = tw2[:kn, kb, (kj * 3 + ki) * 49:
                                       (kj * 3 + ki) * 49 + 49]
                            nc.tensor.matmul(
                                ps[:49, :qn * 64], lhsT, rhs,
                                start=(mm == 0), stop=(mm == 17))
                            mm += 1
                nc.scalar.activation(
                    osb[:, q0 * 64:(q0 + qn) * 64], ps[:49, :qn * 64],
                    AF.Relu, bias=tb2[:], scale=1.0)
            nc.sync.dma_start(out[:], osb[:])

    _legalize_waits(nc)
    return nc


# ---------------------------------------------------------------------------
# Persistent jitted executable (built once at import).
_RT = {}


def _build_runner():
    import jax
    from jax.sharding import Mesh, PartitionSpec
    from jax.experimental.shard_map import shard_map
    from concourse import bass2jax
    from concourse.bass2jax import _bass_exec_p, partition_id_tensor

    nc = _build_nc()
    bass2jax.install_neuronx_cc_hook()

    in_names, out_names, out_avals, zero_outs = [], [], [], []
    partition_name = (nc.partition_id_tensor.name
                      if nc.partition_id_tensor else None)
    for alloc in nc.m.functions[0].allocations:
        if not isinstance(alloc, mybir.MemoryLocationSet):
            continue
        name = alloc.memorylocations[0].name
        if alloc.kind == "ExternalInput":
            if name != partition_name:
                in_names.append(name)
        elif alloc.kind == "ExternalOutput":
            out_names.append(name)
            shape = tuple(alloc.tensor_shape)
            dtype = mybir.dt.np(alloc.dtype)
            out_avals.append(jax.core.ShapedArray(shape, dtype))
            zero_outs.append(np.zeros(shape, dtype))
    n_params = len(in_names)
    all_names = in_names + out_names
    if partition_name is not None:
        all_names.append(partition_name)
    donate = tuple(range(n_params, n_params + len(out_names)))

    def _body(*args):
        operands = list(args)
        if partition_name is not None:
            operands.append(partition_id_tensor())
        outs = _bass_exec_p.bind(
            *operands, out_avals=tuple(out_avals), in_names=tuple(all_names),
            out_names=tuple(out_names), lowering_input_output_aliases=(),
            sim_require_finite=True, sim_require_nnan=True, nc=nc)
        return tuple(outs)

    devices = jax.devices()[:8]
    mesh = Mesh(np.asarray(devices), ("core",))
    nio = n_params + len(out_names)
    sharded = jax.jit(
        shard_map(_body, mesh=mesh,
                  in_specs=(PartitionSpec("core"),) * nio,
                  out_specs=(PartitionSpec("core"),) * len(out_names),
                  check_rep=False),
        donate_argnums=donate, keep_unused=True)
    _RT.update(sharded=sharded, in_names=in_names, out_names=out_names,
               out_avals=out_avals, zero_outs=zero_outs)


def _run(in_maps):
    sharded = _RT["sharded"]
    in_names, zero_outs = _RT["in_names"], _RT["zero_outs"]
    concat_in = [np.concatenate([m[nm] for m in in_maps], axis=0)
                 for nm in in_names]
    concat_zeros = [np.zeros((8 * z.shape[0], *z.shape[1:]), z.dtype)
                    for z in zero_outs]
    out_arrs = sharded(*concat_in, *concat_zeros)
    av = _RT["out_avals"][0]
    return np.asarray(out_arrs[0]).reshape(8, *av.shape)


# ---------------------------------------------------------------------------
# Host-side input prep / output assembly.

def _prep_static(att_w, att_b, agg1_w, agg1_b, agg2_w, agg2_b):
    bf = ml_dtypes.bfloat16
    w7 = np.zeros((3, 128, 49), np.float32)
    b7 = np.zeros((3, 128, 1), np.float32)
    aw = att_w.reshape(289, 49)
    for i in range(3):
        n = min(128, 289 - 128 * i)
        w7[i, :n] = aw[128 * i:128 * i + n]
        b7[i, :n, 0] = att_b[128 * i:128 * i + n]
    w1 = np.zeros((3, 128, 9, 144), np.float32)
    for kb in range(3):
        n = min(128, 289 - 128 * kb)
        w1[kb, :n] = agg1_w[:, 128 * kb:128 * kb + n].reshape(144, n, 9)\
            .transpose(1, 2, 0)
    b1 = np.zeros((2, 128, 1), np.float32)
    b1[0, :, 0] = agg1_b[:128]
    b1[1, :16, 0] = agg1_b[128:]
    w2 = np.zeros((2, 128, 9, 49), np.float32)
    w2[0] = agg2_w[:, :128].reshape(49, 128, 9).transpose(1, 2, 0)
    w2[1, :16] = agg2_w[:, 128:].reshape(49, 16, 9).transpose(1, 2, 0)
    return {
        "w7": w7, "b7": b7,
        "w1": np.ascontiguousarray(w1.reshape(3, 128, 9 * 144)).astype(bf),
        "b1": b1,
        "w2": np.ascontiguousarray(w2.reshape(2, 128, 9 * 49)).astype(bf),
        "b2": agg2_b.reshape(49, 1).astype(np.float32),
        "idn": np.eye(128, dtype=np.float32),
    }


def _prep_inputs(f1, f2, att_w, att_b, agg1_w, agg1_b, agg2_w, agg2_b):
    bf = ml_dtypes.float8_e4m3
    stat = _prep_static(att_w, att_b, agg1_w, agg1_b, agg2_w, agg2_b)
    nrm = np.sqrt(np.einsum('bchw,bchw->bhw', f1, f1))
    alpha = 1.0 / (np.maximum(nrm, 1e-12) * C)
    f1n = (f1 * alpha[:, None]).astype(bf)
    f2p = np.zeros((B_, C, H + 16, W + 16), np.float32)
    f2p[:, :, 8:8 + H, 8:8 + W] = f2
    f2p = f2p.astype(bf)
    in_maps = []
    for core in range(8):
        b, half = core // 2, core % 2
        Bb = half * 32
        f1s = np.zeros((C, F1R, 64), bf)
        lo, hi = Bb - 5, Bb + 37
        clo, chi = max(lo, 0), min(hi, H)
        f1s[:, clo - lo:chi - lo] = f1n[b, :, clo:chi]
        f2s = np.zeros((C, F2R, 80), bf)
        lo2, hi2 = Bb - 13 + 8, Bb + 45 + 8
        clo2, chi2 = max(lo2, 0), min(hi2, H + 16)
        f2s[:, clo2 - lo2:chi2 - lo2] = f2p[b, :, clo2:chi2]
        msk = np.zeros((128, 2), np.float32)
        msk[:, 0] = 0.0 if half == 0 else 1.0
        msk[:, 1] = 1.0 if half == 0 else 0.0
        in_maps.append({
            "f1": np.ascontiguousarray(f1s.reshape(2, 128, F1R * 64)),
            "f2": np.ascontiguousarray(f2s.reshape(2, 128, F2R * 80)),
            "msk": msk, **stat,
        })
    return in_maps


def kernel(f1, f2, att_w, att_b, agg1_w, agg1_b, agg2_w, agg2_b):
    f1 = np.asarray(f1, np.float32)
    f2 = np.asarray(f2, np.float32)
    in_maps = _prep_inputs(
        f1, f2,
        np.asarray(att_w, np.float32), np.asarray(att_b, np.float32),
        np.asarray(agg1_w, np.float32), np.asarray(agg1_b, np.float32),
        np.asarray(agg2_w, np.float32), np.asarray(agg2_b, np.float32))
    res = _run(in_maps)   # [8, 49, 2048] bf16
    out = np.empty((B_, 49, H, W), np.float32)
    for core in range(8):
        b, half = core // 2, core % 2
        out[b, :, half * 32:half * 32 + 32] = \
            res[core].astype(np.float32).reshape(49, 32, 64)
    return out


# ---- compile + warm up at import ------------------------------------------
_build_runner()
_warm = _prep_inputs(
    np.ones((B_, C, H, W), np.float32), np.ones((B_, C, H, W), np.float32),
    np.zeros((289, 1, 7, 7), np.float32), np.zeros(289, np.float32),
    np.zeros((144, 289, 3, 3), np.float32), np.zeros(144, np.float32),
    np.zeros((49, 144, 3, 3), np.float32), np.zeros(49, np.float32))
_run(_warm)
del _warm


# revision 4
# speedup vs baseline: 1.4855x; 1.1399x over previous
"""AttentionCostVolume: Bass/Tile kernel on 8 Trainium2 NeuronCores.

kernel(**inputs) takes the full (unsharded) inputs and returns the full
[4, 49, 64, 64] float32 output.

Sharding: 8 cores = (batch 0..3) x (output-row half), halo-overlapped slices,
no collectives. Per core:
  - cost volume as Gram matmuls: stationary = 128 L2-normalized f1 pixels
    (a row pair), moving = the 18 padded f2 rows they correlate with; the
    [128, 1440] Gram is dumped to scratch DRAM and the 289 diagonals per
    pixel are pulled back with a sheared 3D access pattern, then PE
    transposes put channels on partitions (leaky-relu rides the PSUM->SBUF
    copy).
  - depthwise 7x7 attention conv: 49 per-partition scalar FMA passes on the
    vector engine, f32 accumulation.
  - att_vol = cv * att, then both 3x3 aggregation convs as 9-shift matmul
    accumulations (contraction channel-blocked on partitions), relu+bias on
    the scalar engine.
Everything on-device is bf16 for matmul operands / f32 accumulation.

Compilation and a warm-up execution happen at import; the timed kernel()
call only slices inputs, runs the persistent jitted executable, and
reassembles the output.
"""
import dataclasses
import numpy as np
import ml_dtypes

import concourse.bass as bass
import concourse.tile as tile
from concourse import mybir
from concourse.vector_clock import ScopedClock

bf16 = mybir.dt.bfloat16
f32 = mybir.dt.float32
AF = mybir.ActivationFunctionType

B_, C, H, W = 4, 256, 64, 64
NPAIR = 21
F1R, F2R = 42, 58
OCH = (128, 128, 33)

# ---------------------------------------------------------------------------
# Toolchain workarounds: this walrus build accepts at most ONE sync-wait
# command per instruction; Tile emits more. Spread excess waits onto
# freshly inserted same-engine NOPs placed before the offending instruction.

def _patched_drain_and_barrier(self, tick_clock, wait_clock):
    nc = self.nc
    carrier = nc.sync.nop(nofuse=True)
    wait_clock.add_sem_waits(
        carrier.ins, ScopedClock({None: tick_clock.global_clock}))
    si = carrier.ins.sync_info
    waits = list(si.on_wait) if (si is not None and si.on_wait) else []
    updates = list(si.on_update) if (si is not None and si.on_update) else []
    if len(waits) > 1:
        carrier.ins.sync_info = mybir.SyncInfo(
            on_wait=waits[:1], on_update=updates)
        for w in waits[1:]:
            extra = nc.sync.nop(nofuse=True)
            extra.ins.sync_info = mybir.SyncInfo(on_wait=[w], on_update=[])
    nc.sync.drain()
    nc.all_engine_barrier()
    assert self.sems is not None
    popped = nc._tile_sem_poison_stack.pop()
    assert popped is self._sem_poison
    nc.clear_and_free_semaphores(list(self.sems.allocated().values()))
    nc.all_engine_barrier()


tile.TileContext._drain_and_barrier = _patched_drain_and_barrier

_lgl_uid = [0]


def _legalize_waits(nc):
    for f in nc.m.functions:
        for b in f.blocks:
            out = []
            changed = False
            for inst in b.instructions:
                si = inst.sync_info
                waits = list(si.on_wait) if (si is not None and si.on_wait) else []
                if len(waits) > 1:
                    for w in waits[:-1]:
                        nop = mybir.InstNoOp(
                            name=f"lglnop-{_lgl_uid[0]}", ins=[], outs=[])
                        _lgl_uid[0] += 1
                        nop.engine = inst.engine
                        nop.sync_info = mybir.SyncInfo(on_wait=[w], on_update=[])
                        out.append(nop)
                    updates = list(si.on_update) if si.on_update else []
                    inst.sync_info = mybir.SyncInfo(
                        on_wait=waits[-1:], on_update=updates)
                    changed = True
                out.append(inst)
            if changed:
                b.instructions = out


# ---------------------------------------------------------------------------
def _build_nc():
    nc = bass.Bass("TRN2", target_bir_lowering=False, debug=False,
                   num_devices=8)
    fp8 = mybir.dt.float8e4
    N1, N2 = 2 * 128 * F1R * 64, 2 * 128 * F2R * 80
    NW1, NW2 = 3 * 128 * 9 * 144, 2 * 128 * 9 * 49
    pk8 = nc.declare_dram_parameter("pk8", [N1 + N2], fp8, isOutput=False)
    pkw = nc.declare_dram_parameter("pkw", [NW1 + NW2], bf16, isOutput=False)
    # f32 pack: w7(3*128*49) b7(3*128) b1(2*128) b2(49) msk(128*2) idn(128*128)
    OF = {}
    off = 0
    for nm, sz in (("w7", 3 * 128 * 49), ("b7", 3 * 128), ("b1", 2 * 128),
                   ("b2", 49), ("msk", 128 * 2), ("idn", 128 * 128)):
        OF[nm] = off
        off += sz
    pkf = nc.declare_dram_parameter("pkf", [off], f32, isOutput=False)
    out = nc.declare_dram_parameter("out", [49, 32 * 64], bf16, isOutput=True)

    def pslice(pk, a, p, x):
        return pk[a: a + p * x].rearrange("(p x) -> p x", p=p)
    gdump = nc.dram_tensor("gdump", [NPAIR, 128, 1440], f32)

    with tile.TileContext(nc) as tc:
        with tc.tile_pool(name="big", bufs=1) as big, \
             tc.tile_pool(name="sml", bufs=2) as sml, \
             tc.tile_pool(name="gps", bufs=2, space="PSUM") as gps, \
             tc.tile_pool(name="tps", bufs=1, space="PSUM") as tps, \
             tc.tile_pool(name="aps", bufs=1, space="PSUM") as aps:

            tf18 = big.tile([128, 2, F1R * 64], fp8, tag="tf18")
            for k in range(2):
                nc.sync.dma_start(
                    tf18[:, k], pslice(pk8, k * 128 * F1R * 64, 128, F1R * 64))
            tf28 = big.tile([128, 2, F2R * 80], fp8, tag="tf28")
            for k in range(2):
                nc.sync.dma_start(
                    tf28[:, k],
                    pslice(pk8, N1 + k * 128 * F2R * 80, 128, F2R * 80))
            tf1 = big.tile([128, 2, F1R * 64], bf16, tag="tf1")
            nc.vector.tensor_copy(tf1[:], tf18[:])
            tf2 = big.tile([128, 2, F2R * 80], bf16, tag="tf2")
            nc.vector.tensor_copy(tf2[:], tf28[:])
            tw7 = big.tile([128, 3, 49], f32, tag="tw7")
            tb7 = big.tile([128, 3, 1], f32, tag="tb7")
            for k in range(3):
                nc.sync.dma_start(
                    tw7[:, k], pslice(pkf, OF["w7"] + k * 128 * 49, 128, 49))
                nc.sync.dma_start(
                    tb7[:, k], pslice(pkf, OF["b7"] + k * 128, 128, 1))
            tw1 = big.tile([128, 3, 9 * 144], bf16, tag="tw1")
            for k in range(3):
                nc.sync.dma_start(
                    tw1[:, k], pslice(pkw, k * 128 * 1296, 128, 1296))
            tb1 = big.tile([128, 2, 1], f32, tag="tb1")
            for k in range(2):
                nc.sync.dma_start(
                    tb1[:, k], pslice(pkf, OF["b1"] + k * 128, 128, 1))
            tw2 = big.tile([128, 2, 9 * 49], bf16, tag="tw2")
            for k in range(2):
                nc.sync.dma_start(
                    tw2[:, k], pslice(pkw, NW1 + k * 128 * 441, 128, 441))
            tb2 = big.tile([49, 1], f32, tag="tb2")
            nc.sync.dma_start(tb2[:], pslice(pkf, OF["b2"], 49, 1))
            tmsk = big.tile([128, 2], f32, tag="tmsk")
            nc.sync.dma_start(tmsk[:], pslice(pkf, OF["msk"], 128, 2))
            ident = big.tile([128, 128], f32, tag="ident")
            nc.sync.dma_start(ident[:], pslice(pkf, OF["idn"], 128, 128))

            # cost-volume Grams -> scratch DRAM
            for j in range(NPAIR):
                ps = gps.tile([128, 3, 512], f32, tag="gram")
                for kb in range(2):
                    for k3 in range(3):
                        nc.tensor.matmul(
                            ps[:, k3, :480],
                            tf1[:, kb, j * 128:(j + 1) * 128],
                            tf2[:, kb, j * 160 + 480 * k3:
                                j * 160 + 480 * k3 + 480],
                            start=(kb == 0), stop=(kb == 1))
                gsb = sml.tile([128, 1440], f32, tag="gsb")
                src = dataclasses.replace(
                    ps[:], ap=[[3 * 512, 128], [512, 3], [1, 480]])
                dst = dataclasses.replace(
                    gsb[:], ap=[[1440, 128], [480, 3], [1, 480]])
                nc.scalar.activation(dst, src, AF.Copy)
                nc.sync.dma_start(gdump[j], gsb[:])

            # sheared gather + PE transpose -> cv tiles [o, 42x70] bf16
            cvt = []
            for i in range(3):
                _cv = big.tile([128, 42 * 70], bf16, tag=f"cv{i}", name=f"cv{i}")
                cvt.append(_cv)
            for i in range(3):
                nc.vector.memset(cvt[i][:], 0.0)
            gap = gdump.ap()
            for j in range(NPAIR):
                S = sml.tile([128, 289], f32, tag="S")
                for r in range(2):
                    sap = dataclasses.replace(
                        gap, offset=j * 128 * 1440 + r * 92240,
                        ap=[[1441, 64], [80, 17], [1, 17]])
                    dap = dataclasses.replace(
                        S[:], offset=S[:].offset + r * 289,
                        ap=[[2 * 289, 64], [17, 17], [1, 17]])
                    nc.sync.dma_start(dap, sap)
                for i in range(3):
                    n = OCH[i]
                    tp = tps.tile([128, 128], f32, tag="tp")
                    nc.tensor.transpose(
                        tp[:n, :], S[:, i * 128: i * 128 + n], ident[:])
                    dst = dataclasses.replace(
                        cvt[i][:], offset=cvt[i][:].offset + (2 * j) * 70 + 3,
                        ap=[[42 * 70, n], [70, 2], [1, 64]])
                    src = dataclasses.replace(
                        tp[:], ap=[[128, n], [1, 2], [2, 64]])
                    nc.scalar.activation(dst, src, AF.Lrelu, alpha=0.1)

            # depthwise 7x7 att (f32 acc) + attvol
            avt = []
            for i in range(3):
                _av = big.tile([128, 36 * 66], bf16, tag=f"av{i}", name=f"av{i}")
                avt.append(_av)
            for i in range(3):
                nc.vector.memset(avt[i][:], 0.0)
            for i in range(3):
                n = OCH[i]
                acc = sml.tile([128, 36 * 64], f32, tag="acc")
                tmp = sml.tile([128, 36 * 64], f32, tag="tmp")
                first = True
                for kj in range(7):
                    for ki in range(7):
                        t = kj * 7 + ki
                        src = dataclasses.replace(
                            cvt[i][:n], offset=cvt[i][:].offset + kj * 70 + ki,
                            ap=[[42 * 70, n], [70, 36], [1, 64]])
                        if first:
                            nc.vector.tensor_scalar(
                                acc[:n], src, tw7[:n, i, t:t + 1], tb7[:n, i],
                                op0=mybir.AluOpType.mult,
                                op1=mybir.AluOpType.add)
                            first = False
                        else:
                            nc.vector.tensor_scalar_mul(
                                tmp[:n], src, tw7[:n, i, t:t + 1])
                            nc.vector.tensor_tensor(
                                out=acc[:n], in0=acc[:n], in1=tmp[:n],
                                op=mybir.AluOpType.add)
                cvc = dataclasses.replace(
                    cvt[i][:n], offset=cvt[i][:].offset + 3 * 70 + 3,
                    ap=[[42 * 70, n], [70, 36], [1, 64]])
                avd = dataclasses.replace(
                    avt[i][:n], offset=avt[i][:].offset + 1,
                    ap=[[36 * 66, n], [66, 36], [1, 64]])
                nc.vector.tensor_tensor(
                    out=avd, in0=cvc, in1=acc[:n], op=mybir.AluOpType.mult)

            # agg1 289->144 3x3 (+relu/bias), rows B-1..B+32
            h1a = big.tile([128, 34 * 66], bf16, tag="h1a")
            h1b = big.tile([16, 34 * 66], bf16, tag="h1b")
            h1 = [h1a, h1b]
            for t_ in h1:
                nc.vector.memset(t_[:], 0.0)
            rowblk1 = ((0, 8), (8, 8), (16, 8), (24, 8), (32, 2))
            for ct in range(2):
                co0, con = (0, 128) if ct == 0 else (128, 16)
                for (q0, qn) in rowblk1:
                    ps = aps.tile([128, 512], f32, tag="agg")
                    mm = 0
                    for kb in range(3):
                        n = OCH[kb]
                        for kj in range(3):
                            for ki in range(3):
                                rhs = dataclasses.replace(
                                    avt[kb][:n],
                                    offset=avt[kb][:].offset
                                    + (q0 + kj) * 66 + ki,
                                    ap=[[36 * 66, n], [66, qn], [1, 64]])
                                lhsT = tw1[:n, kb,
                                           (kj * 3 + ki) * 144 + co0:
                                           (kj * 3 + ki) * 144 + co0 + con]
                                nc.tensor.matmul(
                                    ps[:con, :qn * 64], lhsT, rhs,
                                    start=(mm == 0), stop=(mm == 26))
                                mm += 1
                    dst = dataclasses.replace(
                        h1[ct][:con], offset=h1[ct][:].offset + q0 * 66 + 1,
                        ap=[[34 * 66, con], [66, qn], [1, 64]])
                    src = dataclasses.replace(
                        ps[:con], ap=[[512, con], [64, qn], [1, 64]])
                    nc.scalar.activation(dst, src, AF.Relu,
                                         bias=tb1[:con, ct], scale=1.0)
            for ct in range(2):
                con = 128 if ct == 0 else 16
                nc.vector.tensor_scalar_mul(
                    h1[ct][:con, 1:65], h1[ct][:con, 1:65], tmsk[:con, 0:1])
                lo = 33 * 66 + 1
                nc.vector.tensor_scalar_mul(
                    h1[ct][:con, lo:lo + 64], h1[ct][:con, lo:lo + 64],
                    tmsk[:con, 1:2])

            # agg2 144->49 3x3 (+relu/bias), out rows B..B+31
            osb = big.tile([49, 32 * 64], bf16, tag="osb")
            rowblk2 = ((0, 8), (8, 8), (16, 8), (24, 8))
            for (q0, qn) in rowblk2:
                ps = aps.tile([128, 512], f32, tag="agg")
                mm = 0
                for kb in range(2):
                    kn = 128 if kb == 0 else 16
                    for kj in range(3):
                        for ki in range(3):
                            rhs = dataclasses.replace(
                                h1[kb][:kn],
                                offset=h1[kb][:].offset + (q0 + kj) * 66 + ki,
                                ap=[[34 * 66, kn], [66, qn], [1, 64]])
                            lhsT = tw2[:kn, kb, (kj * 3 + ki) * 49:
                                       (kj * 3 + ki) * 49 + 49]
                            nc.tensor.matmul(
                                ps[:49, :qn * 64], lhsT, rhs,
                                start=(mm == 0), stop=(mm == 17))
                            mm += 1
                nc.scalar.activation(
                    osb[:, q0 * 64:(q0 + qn) * 64], ps[:49, :qn * 64],
                    AF.Relu, bias=tb2[:], scale=1.0)
            nc.sync.dma_start(out[:], osb[:])

    _legalize_waits(nc)
    return nc


# ---------------------------------------------------------------------------
# Persistent jitted executable (built once at import).
_RT = {}


def _build_runner():
    import jax
    from jax.sharding import Mesh, PartitionSpec
    from jax.experimental.shard_map import shard_map
    from concourse import bass2jax
    from concourse.bass2jax import _bass_exec_p, partition_id_tensor

    nc = _build_nc()
    bass2jax.install_neuronx_cc_hook()

    in_names, out_names, out_avals, zero_outs = [], [], [], []
    partition_name = (nc.partition_id_tensor.name
                      if nc.partition_id_tensor else None)
    for alloc in nc.m.functions[0].allocations:
        if not isinstance(alloc, mybir.MemoryLocationSet):
            continue
        name = alloc.memorylocations[0].name
        if alloc.kind == "ExternalInput":
            if name != partition_name:
                in_names.append(name)
        elif alloc.kind == "ExternalOutput":
            out_names.append(name)
            shape = tuple(alloc.tensor_shape)
            dtype = mybir.dt.np(alloc.dtype)
            out_avals.append(jax.core.ShapedArray(shape, dtype))
            zero_outs.append(np.zeros(shape, dtype))
    n_params = len(in_names)
    all_names = in_names + out_names
    if partition_name is not None:
        all_names.append(partition_name)
    donate = tuple(range(n_params, n_params + len(out_names)))

    def _body(*args):
        operands = list(args)
        if partition_name is not None:
            operands.append(partition_id_tensor())
        outs = _bass_exec_p.bind(
            *operands, out_avals=tuple(out_avals), in_names=tuple(all_names),
            out_names=tuple(out_names), lowering_input_output_aliases=(),
            sim_require_finite=True, sim_require_nnan=True, nc=nc)
        return tuple(outs)

    devices = jax.devices()[:8]
    mesh = Mesh(np.asarray(devices), ("core",))
    nio = n_params + len(out_names)
    sharded = jax.jit(
        shard_map(_body, mesh=mesh,
                  in_specs=(PartitionSpec("core"),) * nio,
                  out_specs=(PartitionSpec("core"),) * len(out_names),
                  check_rep=False),
        donate_argnums=donate, keep_unused=True)
    _RT.update(sharded=sharded, in_names=in_names, out_names=out_names,
               out_avals=out_avals, zero_outs=zero_outs)


def _run(in_maps):
    sharded = _RT["sharded"]
    in_names, zero_outs = _RT["in_names"], _RT["zero_outs"]
    concat_in = [np.concatenate([m[nm] for m in in_maps], axis=0)
                 for nm in in_names]
    concat_zeros = [np.zeros((8 * z.shape[0], *z.shape[1:]), z.dtype)
                    for z in zero_outs]
    out_arrs = sharded(*concat_in, *concat_zeros)
    av = _RT["out_avals"][0]
    return np.asarray(out_arrs[0]).reshape(8, *av.shape)


# ---------------------------------------------------------------------------
# Host-side input prep / output assembly.

def _prep_static(att_w, att_b, agg1_w, agg1_b, agg2_w, agg2_b):
    bf = ml_dtypes.bfloat16
    w7 = np.zeros((3, 128, 49), np.float32)
    b7 = np.zeros((3, 128, 1), np.float32)
    aw = att_w.reshape(289, 49)
    for i in range(3):
        n = min(128, 289 - 128 * i)
        w7[i, :n] = aw[128 * i:128 * i + n]
        b7[i, :n, 0] = att_b[128 * i:128 * i + n]
    w1 = np.zeros((3, 128, 9, 144), np.float32)
    for kb in range(3):
        n = min(128, 289 - 128 * kb)
        w1[kb, :n] = agg1_w[:, 128 * kb:128 * kb + n].reshape(144, n, 9)\
            .transpose(1, 2, 0)
    b1 = np.zeros((2, 128, 1), np.float32)
    b1[0, :, 0] = agg1_b[:128]
    b1[1, :16, 0] = agg1_b[128:]
    w2 = np.zeros((2, 128, 9, 49), np.float32)
    w2[0] = agg2_w[:, :128].reshape(49, 128, 9).transpose(1, 2, 0)
    w2[1, :16] = agg2_w[:, 128:].reshape(49, 16, 9).transpose(1, 2, 0)
    pkw = np.concatenate([
        np.ascontiguousarray(w1).astype(ml_dtypes.bfloat16).ravel(),
        np.ascontiguousarray(w2).astype(ml_dtypes.bfloat16).ravel()])
    pkf_base = np.concatenate([
        w7.ravel(), b7.ravel(), b1.ravel(),
        agg2_b.astype(np.float32).ravel()])
    return pkw, pkf_base


def _prep_inputs(f1, f2, att_w, att_b, agg1_w, agg1_b, agg2_w, agg2_b):
    bf = ml_dtypes.float8_e4m3
    pkw, pkf_base = _prep_static(att_w, att_b, agg1_w, agg1_b, agg2_w, agg2_b)
    idn = np.eye(128, dtype=np.float32).ravel()
    nrm = np.sqrt(np.einsum('bchw,bchw->bhw', f1, f1))
    alpha = 1.0 / (np.maximum(nrm, 1e-12) * C)
    f1n = (f1 * alpha[:, None]).astype(bf)
    f2p = np.zeros((B_, C, H + 16, W + 16), np.float32)
    f2p[:, :, 8:8 + H, 8:8 + W] = f2
    f2p = f2p.astype(bf)
    in_maps = []
    for core in range(8):
        b, half = core // 2, core % 2
        Bb = half * 32
        f1s = np.zeros((C, F1R, 64), bf)
        lo, hi = Bb - 5, Bb + 37
        clo, chi = max(lo, 0), min(hi, H)
        f1s[:, clo - lo:chi - lo] = f1n[b, :, clo:chi]
        f2s = np.zeros((C, F2R, 80), bf)
        lo2, hi2 = Bb - 13 + 8, Bb + 45 + 8
        clo2, chi2 = max(lo2, 0), min(hi2, H + 16)
        f2s[:, clo2 - lo2:chi2 - lo2] = f2p[b, :, clo2:chi2]
        msk = np.zeros((128, 2), np.float32)
        msk[:, 0] = 0.0 if half == 0 else 1.0
        msk[:, 1] = 1.0 if half == 0 else 0.0
        pk8 = np.concatenate([f1s.ravel(), f2s.ravel()])
        pkf = np.concatenate([pkf_base, msk.ravel(), idn])
        in_maps.append({"pk8": pk8, "pkw": pkw, "pkf": pkf})
    return in_maps


def kernel(f1, f2, att_w, att_b, agg1_w, agg1_b, agg2_w, agg2_b):
    f1 = np.asarray(f1, np.float32)
    f2 = np.asarray(f2, np.float32)
    in_maps = _prep_inputs(
        f1, f2,
        np.asarray(att_w, np.float32), np.asarray(att_b, np.float32),
        np.asarray(agg1_w, np.float32), np.asarray(agg1_b, np.float32),
        np.asarray(agg2_w, np.float32), np.asarray(agg2_b, np.float32))
    res = _run(in_maps)   # [8, 49, 2048] bf16
    out = np.empty((B_, 49, H, W), np.float32)
    for core in range(8):
        b, half = core // 2, core % 2
        out[b, :, half * 32:half * 32 + 32] = \
            res[core].astype(np.float32).reshape(49, 32, 64)
    return out


# ---- compile + warm up at import ------------------------------------------
_build_runner()
_warm = _prep_inputs(
    np.ones((B_, C, H, W), np.float32), np.ones((B_, C, H, W), np.float32),
    np.zeros((289, 1, 7, 7), np.float32), np.zeros(289, np.float32),
    np.zeros((144, 289, 3, 3), np.float32), np.zeros(144, np.float32),
    np.zeros((49, 144, 3, 3), np.float32), np.zeros(49, np.float32))
_run(_warm)
del _warm


# revision 5
# speedup vs baseline: 1.5603x; 1.0504x over previous
"""AttentionCostVolume: Bass/Tile kernel on 8 Trainium2 NeuronCores.

kernel(**inputs) takes the full (unsharded) inputs and returns the full
[4, 49, 64, 64] float32 output.

Sharding: 8 cores = (batch 0..3) x (output-row half), halo-overlapped slices,
no collectives. Per core:
  - cost volume as Gram matmuls: stationary = 128 L2-normalized f1 pixels
    (a row pair), moving = the 18 padded f2 rows they correlate with; the
    [128, 1440] Gram is dumped to scratch DRAM and the 289 diagonals per
    pixel are pulled back with a sheared 3D access pattern, then PE
    transposes put channels on partitions (leaky-relu rides the PSUM->SBUF
    copy).
  - depthwise 7x7 attention conv: 49 per-partition scalar FMA passes on the
    vector engine, f32 accumulation.
  - att_vol = cv * att, then both 3x3 aggregation convs as 9-shift matmul
    accumulations (contraction channel-blocked on partitions), relu+bias on
    the scalar engine.
Everything on-device is bf16 for matmul operands / f32 accumulation.

Compilation and a warm-up execution happen at import; the timed kernel()
call only slices inputs, runs the persistent jitted executable, and
reassembles the output.
"""
import dataclasses
import numpy as np
import ml_dtypes

import concourse.bass as bass
import concourse.tile as tile
from concourse import mybir
from concourse.vector_clock import ScopedClock

bf16 = mybir.dt.bfloat16
f32 = mybir.dt.float32
AF = mybir.ActivationFunctionType

B_, C, H, W = 4, 256, 64, 64
NPAIR = 21
F1R, F2R = 42, 58
OCH = (128, 128, 33)

# ---------------------------------------------------------------------------
# Toolchain workarounds: this walrus build accepts at most ONE sync-wait
# command per instruction; Tile emits more. Spread excess waits onto
# freshly inserted same-engine NOPs placed before the offending instruction.

def _patched_drain_and_barrier(self, tick_clock, wait_clock):
    nc = self.nc
    carrier = nc.sync.nop(nofuse=True)
    wait_clock.add_sem_waits(
        carrier.ins, ScopedClock({None: tick_clock.global_clock}))
    si = carrier.ins.sync_info
    waits = list(si.on_wait) if (si is not None and si.on_wait) else []
    updates = list(si.on_update) if (si is not None and si.on_update) else []
    if len(waits) > 1:
        carrier.ins.sync_info = mybir.SyncInfo(
            on_wait=waits[:1], on_update=updates)
        for w in waits[1:]:
            extra = nc.sync.nop(nofuse=True)
            extra.ins.sync_info = mybir.SyncInfo(on_wait=[w], on_update=[])
    nc.sync.drain()
    nc.all_engine_barrier()
    assert self.sems is not None
    popped = nc._tile_sem_poison_stack.pop()
    assert popped is self._sem_poison
    nc.clear_and_free_semaphores(list(self.sems.allocated().values()))
    nc.all_engine_barrier()


tile.TileContext._drain_and_barrier = _patched_drain_and_barrier

_lgl_uid = [0]


def _legalize_waits(nc):
    for f in nc.m.functions:
        for b in f.blocks:
            out = []
            changed = False
            for inst in b.instructions:
                si = inst.sync_info
                waits = list(si.on_wait) if (si is not None and si.on_wait) else []
                if len(waits) > 1:
                    for w in waits[:-1]:
                        nop = mybir.InstNoOp(
                            name=f"lglnop-{_lgl_uid[0]}", ins=[], outs=[])
                        _lgl_uid[0] += 1
                        nop.engine = inst.engine
                        nop.sync_info = mybir.SyncInfo(on_wait=[w], on_update=[])
                        out.append(nop)
                    updates = list(si.on_update) if si.on_update else []
                    inst.sync_info = mybir.SyncInfo(
                        on_wait=waits[-1:], on_update=updates)
                    changed = True
                out.append(inst)
            if changed:
                b.instructions = out


# ---------------------------------------------------------------------------
def _build_nc():
    nc = bass.Bass("TRN2", target_bir_lowering=False, debug=False,
                   num_devices=8)
    fp8 = mybir.dt.float8e4
    N1, N2 = 2 * 128 * F1R * 64, 2 * 128 * F2R * 80
    NW1, NW2 = 3 * 128 * 9 * 144, 2 * 128 * 9 * 49
    pk8 = nc.declare_dram_parameter("pk8", [N1 + N2 + NW1], fp8, isOutput=False)
    pkw = nc.declare_dram_parameter("pkw", [NW2], bf16, isOutput=False)
    # f32 pack: w7(3*128*49) b7(3*128) b1(2*128) b2(49) msk(128*2) idn(128*128)
    OF = {}
    off = 0
    for nm, sz in (("w7", 3 * 128 * 49), ("b7", 3 * 128), ("b1", 2 * 128),
                   ("b2", 49), ("msk", 128 * 2), ("idn", 128 * 128)):
        OF[nm] = off
        off += sz
    pkf = nc.declare_dram_parameter("pkf", [off], f32, isOutput=False)
    out = nc.declare_dram_parameter("out", [49, 32 * 64], bf16, isOutput=True)

    def pslice(pk, a, p, x):
        return pk[a: a + p * x].rearrange("(p x) -> p x", p=p)
    gdump = nc.dram_tensor("gdump", [NPAIR, 128, 1440], f32)

    with tile.TileContext(nc) as tc:
        with tc.tile_pool(name="big", bufs=1) as big, \
             tc.tile_pool(name="sml", bufs=2) as sml, \
             tc.tile_pool(name="gps", bufs=2, space="PSUM") as gps, \
             tc.tile_pool(name="tps", bufs=1, space="PSUM") as tps, \
             tc.tile_pool(name="aps", bufs=1, space="PSUM") as aps:

            tf18 = big.tile([128, 2, F1R * 64], fp8, tag="tf18")
            for k in range(2):
                nc.sync.dma_start(
                    tf18[:, k], pslice(pk8, k * 128 * F1R * 64, 128, F1R * 64))
            tf28 = big.tile([128, 2, F2R * 80], fp8, tag="tf28")
            for k in range(2):
                nc.sync.dma_start(
                    tf28[:, k],
                    pslice(pk8, N1 + k * 128 * F2R * 80, 128, F2R * 80))
            tf1 = big.tile([128, 2, F1R * 64], bf16, tag="tf1")
            nc.vector.tensor_copy(tf1[:], tf18[:])
            tf2 = big.tile([128, 2, F2R * 80], bf16, tag="tf2")
            nc.vector.tensor_copy(tf2[:], tf28[:])
            tw7 = big.tile([128, 3, 49], f32, tag="tw7")
            tb7 = big.tile([128, 3, 1], f32, tag="tb7")
            for k in range(3):
                nc.sync.dma_start(
                    tw7[:, k], pslice(pkf, OF["w7"] + k * 128 * 49, 128, 49))
                nc.sync.dma_start(
                    tb7[:, k], pslice(pkf, OF["b7"] + k * 128, 128, 1))
            tw18 = big.tile([128, 3, 9 * 144], fp8, tag="tw18")
            for k in range(3):
                nc.sync.dma_start(
                    tw18[:, k],
                    pslice(pk8, N1 + N2 + k * 128 * 1296, 128, 1296))
            tw1 = big.tile([128, 3, 9 * 144], bf16, tag="tw1")
            nc.vector.tensor_copy(tw1[:], tw18[:])
            tb1 = big.tile([128, 2, 1], f32, tag="tb1")
            for k in range(2):
                nc.sync.dma_start(
                    tb1[:, k], pslice(pkf, OF["b1"] + k * 128, 128, 1))
            tw2 = big.tile([128, 2, 9 * 49], bf16, tag="tw2")
            for k in range(2):
                nc.sync.dma_start(
                    tw2[:, k], pslice(pkw, k * 128 * 441, 128, 441))
            tb2 = big.tile([49, 1], f32, tag="tb2")
            nc.sync.dma_start(tb2[:], pslice(pkf, OF["b2"], 49, 1))
            tmsk = big.tile([128, 2], f32, tag="tmsk")
            nc.sync.dma_start(tmsk[:], pslice(pkf, OF["msk"], 128, 2))
            ident = big.tile([128, 128], f32, tag="ident")
            nc.sync.dma_start(ident[:], pslice(pkf, OF["idn"], 128, 128))

            # cost-volume Grams -> scratch DRAM
            for j in range(NPAIR):
                ps = gps.tile([128, 3, 512], f32, tag="gram")
                for kb in range(2):
                    for k3 in range(3):
                        nc.tensor.matmul(
                            ps[:, k3, :480],
                            tf1[:, kb, j * 128:(j + 1) * 128],
                            tf2[:, kb, j * 160 + 480 * k3:
                                j * 160 + 480 * k3 + 480],
                            start=(kb == 0), stop=(kb == 1))
                gsb = sml.tile([128, 1440], f32, tag="gsb")
                src = dataclasses.replace(
                    ps[:], ap=[[3 * 512, 128], [512, 3], [1, 480]])
                dst = dataclasses.replace(
                    gsb[:], ap=[[1440, 128], [480, 3], [1, 480]])
                nc.scalar.activation(dst, src, AF.Copy)
                nc.sync.dma_start(gdump[j], gsb[:])

            # sheared gather + PE transpose -> cv tiles [o, 42x70] bf16
            cvt = []
            for i in range(3):
                _cv = big.tile([128, 42 * 70], bf16, tag=f"cv{i}", name=f"cv{i}")
                cvt.append(_cv)
            for i in range(3):
                nc.vector.memset(cvt[i][:], 0.0)
            gap = gdump.ap()
            for j in range(NPAIR):
                S = sml.tile([128, 289], f32, tag="S")
                for r in range(2):
                    sap = dataclasses.replace(
                        gap, offset=j * 128 * 1440 + r * 92240,
                        ap=[[1441, 64], [80, 17], [1, 17]])
                    dap = dataclasses.replace(
                        S[:], offset=S[:].offset + r * 289,
                        ap=[[2 * 289, 64], [17, 17], [1, 17]])
                    nc.sync.dma_start(dap, sap)
                for i in range(3):
                    n = OCH[i]
                    tp = tps.tile([128, 128], f32, tag="tp")
                    nc.tensor.transpose(
                        tp[:n, :], S[:, i * 128: i * 128 + n], ident[:])
                    dst = dataclasses.replace(
                        cvt[i][:], offset=cvt[i][:].offset + (2 * j) * 70 + 3,
                        ap=[[42 * 70, n], [70, 2], [1, 64]])
                    src = dataclasses.replace(
                        tp[:], ap=[[128, n], [1, 2], [2, 64]])
                    nc.scalar.activation(dst, src, AF.Lrelu, alpha=0.1)

            # depthwise 7x7 att (f32 acc) + attvol
            avt = []
            for i in range(3):
                _av = big.tile([128, 36 * 66], bf16, tag=f"av{i}", name=f"av{i}")
                avt.append(_av)
            for i in range(3):
                nc.vector.memset(avt[i][:], 0.0)
            for i in range(3):
                n = OCH[i]
                acc = sml.tile([128, 36 * 64], f32, tag="acc")
                tmp = sml.tile([128, 36 * 64], f32, tag="tmp")
                first = True
                for kj in range(7):
                    for ki in range(7):
                        t = kj * 7 + ki
                        src = dataclasses.replace(
                            cvt[i][:n], offset=cvt[i][:].offset + kj * 70 + ki,
                            ap=[[42 * 70, n], [70, 36], [1, 64]])
                        if first:
                            nc.vector.tensor_scalar(
                                acc[:n], src, tw7[:n, i, t:t + 1], tb7[:n, i],
                                op0=mybir.AluOpType.mult,
                                op1=mybir.AluOpType.add)
                            first = False
                        else:
                            nc.vector.tensor_scalar_mul(
                                tmp[:n], src, tw7[:n, i, t:t + 1])
                            nc.vector.tensor_tensor(
                                out=acc[:n], in0=acc[:n], in1=tmp[:n],
                                op=mybir.AluOpType.add)
                cvc = dataclasses.replace(
                    cvt[i][:n], offset=cvt[i][:].offset + 3 * 70 + 3,
                    ap=[[42 * 70, n], [70, 36], [1, 64]])
                avd = dataclasses.replace(
                    avt[i][:n], offset=avt[i][:].offset + 1,
                    ap=[[36 * 66, n], [66, 36], [1, 64]])
                nc.vector.tensor_tensor(
                    out=avd, in0=cvc, in1=acc[:n], op=mybir.AluOpType.mult)

            # agg1 289->144 3x3 (+relu/bias), rows B-1..B+32
            h1a = big.tile([128, 34 * 66], bf16, tag="h1a")
            h1b = big.tile([16, 34 * 66], bf16, tag="h1b")
            h1 = [h1a, h1b]
            for t_ in h1:
                nc.vector.memset(t_[:], 0.0)
            rowblk1 = ((0, 8), (8, 8), (16, 8), (24, 8), (32, 2))
            for ct in range(2):
                co0, con = (0, 128) if ct == 0 else (128, 16)
                for (q0, qn) in rowblk1:
                    ps = aps.tile([128, 512], f32, tag="agg")
                    mm = 0
                    for kb in range(3):
                        n = OCH[kb]
                        for kj in range(3):
                            for ki in range(3):
                                rhs = dataclasses.replace(
                                    avt[kb][:n],
                                    offset=avt[kb][:].offset
                                    + (q0 + kj) * 66 + ki,
                                    ap=[[36 * 66, n], [66, qn], [1, 64]])
                                lhsT = tw1[:n, kb,
                                           (kj * 3 + ki) * 144 + co0:
                                           (kj * 3 + ki) * 144 + co0 + con]
                                nc.tensor.matmul(
                                    ps[:con, :qn * 64], lhsT, rhs,
                                    start=(mm == 0), stop=(mm == 26))
                                mm += 1
                    dst = dataclasses.replace(
                        h1[ct][:con], offset=h1[ct][:].offset + q0 * 66 + 1,
                        ap=[[34 * 66, con], [66, qn], [1, 64]])
                    src = dataclasses.replace(
                        ps[:con], ap=[[512, con], [64, qn], [1, 64]])
                    nc.scalar.activation(dst, src, AF.Relu,
                                         bias=tb1[:con, ct], scale=1.0)
            for ct in range(2):
                con = 128 if ct == 0 else 16
                nc.vector.tensor_scalar_mul(
                    h1[ct][:con, 1:65], h1[ct][:con, 1:65], tmsk[:con, 0:1])
                lo = 33 * 66 + 1
                nc.vector.tensor_scalar_mul(
                    h1[ct][:con, lo:lo + 64], h1[ct][:con, lo:lo + 64],
                    tmsk[:con, 1:2])

            # agg2 144->49 3x3 (+relu/bias), out rows B..B+31
            osb = big.tile([49, 32 * 64], bf16, tag="osb")
            rowblk2 = ((0, 8), (8, 8), (16, 8), (24, 8))
            for (q0, qn) in rowblk2:
                ps = aps.tile([128, 512], f32, tag="agg")
                mm = 0
                for kb in range(2):
                    kn = 128 if kb == 0 else 16
                    for kj in range(3):
                        for ki in range(3):
                            rhs = dataclasses.replace(
                                h1[kb][:kn],
                                offset=h1[kb][:].offset + (q0 + kj) * 66 + ki,
                                ap=[[34 * 66, kn], [66, qn], [1, 64]])
                            lhsT = tw2[:kn, kb, (kj * 3 + ki) * 49:
                                       (kj * 3 + ki) * 49 + 49]
                            nc.tensor.matmul(
                                ps[:49, :qn * 64], lhsT, rhs,
                                start=(mm == 0), stop=(mm == 17))
                            mm += 1
                nc.scalar.activation(
                    osb[:, q0 * 64:(q0 + qn) * 64], ps[:49, :qn * 64],
                    AF.Relu, bias=tb2[:], scale=1.0)
            nc.sync.dma_start(out[:], osb[:])

    _legalize_waits(nc)
    return nc


# ---------------------------------------------------------------------------
# Persistent jitted executable (built once at import).
_RT = {}


def _build_runner():
    import jax
    from jax.sharding import Mesh, PartitionSpec
    from jax.experimental.shard_map import shard_map
    from concourse import bass2jax
    from concourse.bass2jax import _bass_exec_p, partition_id_tensor

    nc = _build_nc()
    bass2jax.install_neuronx_cc_hook()

    in_names, out_names, out_avals, zero_outs = [], [], [], []
    partition_name = (nc.partition_id_tensor.name
                      if nc.partition_id_tensor else None)
    for alloc in nc.m.functions[0].allocations:
        if not isinstance(alloc, mybir.MemoryLocationSet):
            continue
        name = alloc.memorylocations[0].name
        if alloc.kind == "ExternalInput":
            if name != partition_name:
                in_names.append(name)
        elif alloc.kind == "ExternalOutput":
            out_names.append(name)
            shape = tuple(alloc.tensor_shape)
            dtype = mybir.dt.np(alloc.dtype)
            out_avals.append(jax.core.ShapedArray(shape, dtype))
            zero_outs.append(np.zeros(shape, dtype))
    n_params = len(in_names)
    all_names = in_names + out_names
    if partition_name is not None:
        all_names.append(partition_name)
    donate = tuple(range(n_params, n_params + len(out_names)))

    def _body(*args):
        operands = list(args)
        if partition_name is not None:
            operands.append(partition_id_tensor())
        outs = _bass_exec_p.bind(
            *operands, out_avals=tuple(out_avals), in_names=tuple(all_names),
            out_names=tuple(out_names), lowering_input_output_aliases=(),
            sim_require_finite=True, sim_require_nnan=True, nc=nc)
        return tuple(outs)

    devices = jax.devices()[:8]
    mesh = Mesh(np.asarray(devices), ("core",))
    nio = n_params + len(out_names)
    sharded = jax.jit(
        shard_map(_body, mesh=mesh,
                  in_specs=(PartitionSpec("core"),) * nio,
                  out_specs=(PartitionSpec("core"),) * len(out_names),
                  check_rep=False),
        donate_argnums=donate, keep_unused=True)
    _RT.update(sharded=sharded, in_names=in_names, out_names=out_names,
               out_avals=out_avals, zero_outs=zero_outs)


def _run(in_maps):
    sharded = _RT["sharded"]
    in_names, zero_outs = _RT["in_names"], _RT["zero_outs"]
    concat_in = [np.concatenate([m[nm] for m in in_maps], axis=0)
                 for nm in in_names]
    concat_zeros = [np.zeros((8 * z.shape[0], *z.shape[1:]), z.dtype)
                    for z in zero_outs]
    out_arrs = sharded(*concat_in, *concat_zeros)
    av = _RT["out_avals"][0]
    return np.asarray(out_arrs[0]).reshape(8, *av.shape)


# ---------------------------------------------------------------------------
# Host-side input prep / output assembly.

def _prep_static(att_w, att_b, agg1_w, agg1_b, agg2_w, agg2_b):
    bf = ml_dtypes.bfloat16
    w7 = np.zeros((3, 128, 49), np.float32)
    b7 = np.zeros((3, 128, 1), np.float32)
    aw = att_w.reshape(289, 49)
    for i in range(3):
        n = min(128, 289 - 128 * i)
        w7[i, :n] = aw[128 * i:128 * i + n]
        b7[i, :n, 0] = att_b[128 * i:128 * i + n]
    w1 = np.zeros((3, 128, 9, 144), np.float32)
    for kb in range(3):
        n = min(128, 289 - 128 * kb)
        w1[kb, :n] = agg1_w[:, 128 * kb:128 * kb + n].reshape(144, n, 9)\
            .transpose(1, 2, 0)
    b1 = np.zeros((2, 128, 1), np.float32)
    b1[0, :, 0] = agg1_b[:128]
    b1[1, :16, 0] = agg1_b[128:]
    w2 = np.zeros((2, 128, 9, 49), np.float32)
    w2[0] = agg2_w[:, :128].reshape(49, 128, 9).transpose(1, 2, 0)
    w2[1, :16] = agg2_w[:, 128:].reshape(49, 16, 9).transpose(1, 2, 0)
    w1f8 = np.ascontiguousarray(w1).astype(ml_dtypes.float8_e4m3).ravel()
    pkw = np.ascontiguousarray(w2).astype(ml_dtypes.bfloat16).ravel()
    pkf_base = np.concatenate([
        w7.ravel(), b7.ravel(), b1.ravel(),
        agg2_b.astype(np.float32).ravel()])
    return pkw, pkf_base, w1f8


def _prep_inputs(f1, f2, att_w, att_b, agg1_w, agg1_b, agg2_w, agg2_b):
    bf = ml_dtypes.float8_e4m3
    pkw, pkf_base, w1f8 = _prep_static(
        att_w, att_b, agg1_w, agg1_b, agg2_w, agg2_b)
    idn = np.eye(128, dtype=np.float32).ravel()
    nrm = np.sqrt(np.einsum('bchw,bchw->bhw', f1, f1))
    alpha = 1.0 / (np.maximum(nrm, 1e-12) * C)
    f1n = (f1 * alpha[:, None]).astype(bf)
    f2p = np.zeros((B_, C, H + 16, W + 16), np.float32)
    f2p[:, :, 8:8 + H, 8:8 + W] = f2
    f2p = f2p.astype(bf)
    in_maps = []
    for core in range(8):
        b, half = core // 2, core % 2
        Bb = half * 32
        f1s = np.zeros((C, F1R, 64), bf)
        lo, hi = Bb - 5, Bb + 37
        clo, chi = max(lo, 0), min(hi, H)
        f1s[:, clo - lo:chi - lo] = f1n[b, :, clo:chi]
        f2s = np.zeros((C, F2R, 80), bf)
        lo2, hi2 = Bb - 13 + 8, Bb + 45 + 8
        clo2, chi2 = max(lo2, 0), min(hi2, H + 16)
        f2s[:, clo2 - lo2:chi2 - lo2] = f2p[b, :, clo2:chi2]
        msk = np.zeros((128, 2), np.float32)
        msk[:, 0] = 0.0 if half == 0 else 1.0
        msk[:, 1] = 1.0 if half == 0 else 0.0
        pk8 = np.concatenate([f1s.ravel(), f2s.ravel(), w1f8])
        pkf = np.concatenate([pkf_base, msk.ravel(), idn])
        in_maps.append({"pk8": pk8, "pkw": pkw, "pkf": pkf})
    return in_maps


def kernel(f1, f2, att_w, att_b, agg1_w, agg1_b, agg2_w, agg2_b):
    f1 = np.asarray(f1, np.float32)
    f2 = np.asarray(f2, np.float32)
    in_maps = _prep_inputs(
        f1, f2,
        np.asarray(att_w, np.float32), np.asarray(att_b, np.float32),
        np.asarray(agg1_w, np.float32), np.asarray(agg1_b, np.float32),
        np.asarray(agg2_w, np.float32), np.asarray(agg2_b, np.float32))
    res = _run(in_maps)   # [8, 49, 2048] bf16
    out = np.empty((B_, 49, H, W), np.float32)
    for core in range(8):
        b, half = core // 2, core % 2
        out[b, :, half * 32:half * 32 + 32] = \
            res[core].astype(np.float32).reshape(49, 32, 64)
    return out


# ---- compile + warm up at import ------------------------------------------
_build_runner()
_warm = _prep_inputs(
    np.ones((B_, C, H, W), np.float32), np.ones((B_, C, H, W), np.float32),
    np.zeros((289, 1, 7, 7), np.float32), np.zeros(289, np.float32),
    np.zeros((144, 289, 3, 3), np.float32), np.zeros(144, np.float32),
    np.zeros((49, 144, 3, 3), np.float32), np.zeros(49, np.float32))
_run(_warm)
del _warm


# revision 6
# speedup vs baseline: 1.7579x; 1.1266x over previous
"""AttentionCostVolume: Bass/Tile kernel on 8 Trainium2 NeuronCores.

kernel(**inputs) takes the full (unsharded) inputs and returns the full
[4, 49, 64, 64] float32 output.

Sharding: 8 cores = (batch 0..3) x (output-row half), halo-overlapped slices,
no collectives. Per core:
  - cost volume as Gram matmuls: stationary = 128 L2-normalized f1 pixels
    (a row pair), moving = the 18 padded f2 rows they correlate with; the
    [128, 1440] Gram is dumped to scratch DRAM and the 289 diagonals per
    pixel are pulled back with a sheared 3D access pattern, then PE
    transposes put channels on partitions (leaky-relu rides the PSUM->SBUF
    copy).
  - depthwise 7x7 attention conv: 49 per-partition scalar FMA passes on the
    vector engine, f32 accumulation.
  - att_vol = cv * att, then both 3x3 aggregation convs as 9-shift matmul
    accumulations (contraction channel-blocked on partitions), relu+bias on
    the scalar engine.
Everything on-device is bf16 for matmul operands / f32 accumulation.

Compilation and a warm-up execution happen at import; the timed kernel()
call only slices inputs, runs the persistent jitted executable, and
reassembles the output.
"""
import dataclasses
import numpy as np
import ml_dtypes

import concourse.bass as bass
import concourse.tile as tile
from concourse import mybir
from concourse.vector_clock import ScopedClock

bf16 = mybir.dt.bfloat16
f32 = mybir.dt.float32
AF = mybir.ActivationFunctionType

B_, C, H, W = 4, 256, 64, 64
NPAIR = 21
F1R, F2R = 42, 58
OCH = (128, 128, 33)

# ---------------------------------------------------------------------------
# Toolchain workarounds: this walrus build accepts at most ONE sync-wait
# command per instruction; Tile emits more. Spread excess waits onto
# freshly inserted same-engine NOPs placed before the offending instruction.

def _patched_drain_and_barrier(self, tick_clock, wait_clock):
    nc = self.nc
    carrier = nc.sync.nop(nofuse=True)
    wait_clock.add_sem_waits(
        carrier.ins, ScopedClock({None: tick_clock.global_clock}))
    si = carrier.ins.sync_info
    waits = list(si.on_wait) if (si is not None and si.on_wait) else []
    updates = list(si.on_update) if (si is not None and si.on_update) else []
    if len(waits) > 1:
        carrier.ins.sync_info = mybir.SyncInfo(
            on_wait=waits[:1], on_update=updates)
        for w in waits[1:]:
            extra = nc.sync.nop(nofuse=True)
            extra.ins.sync_info = mybir.SyncInfo(on_wait=[w], on_update=[])
    nc.sync.drain()
    nc.all_engine_barrier()
    assert self.sems is not None
    popped = nc._tile_sem_poison_stack.pop()
    assert popped is self._sem_poison
    nc.clear_and_free_semaphores(list(self.sems.allocated().values()))
    nc.all_engine_barrier()


tile.TileContext._drain_and_barrier = _patched_drain_and_barrier

_lgl_uid = [0]


def _legalize_waits(nc):
    for f in nc.m.functions:
        for b in f.blocks:
            out = []
            changed = False
            for inst in b.instructions:
                si = inst.sync_info
                waits = list(si.on_wait) if (si is not None and si.on_wait) else []
                if len(waits) > 1:
                    for w in waits[:-1]:
                        nop = mybir.InstNoOp(
                            name=f"lglnop-{_lgl_uid[0]}", ins=[], outs=[])
                        _lgl_uid[0] += 1
                        nop.engine = inst.engine
                        nop.sync_info = mybir.SyncInfo(on_wait=[w], on_update=[])
                        out.append(nop)
                    updates = list(si.on_update) if si.on_update else []
                    inst.sync_info = mybir.SyncInfo(
                        on_wait=waits[-1:], on_update=updates)
                    changed = True
                out.append(inst)
            if changed:
                b.instructions = out


# ---------------------------------------------------------------------------
def _build_nc():
    nc = bass.Bass("TRN2", target_bir_lowering=False, debug=False,
                   num_devices=8)
    fp8 = mybir.dt.float8e4
    N1, N2 = 2 * 128 * F1R * 64, 2 * 128 * F2R * 64
    NW1, NW2 = 3 * 128 * 9 * 144, 2 * 128 * 9 * 49
    pk8 = nc.declare_dram_parameter("pk8", [N1 + N2 + NW1], fp8, isOutput=False)
    pkw = nc.declare_dram_parameter("pkw", [NW2], bf16, isOutput=False)
    # f32 pack: w7(3*128*49) b7(3*128) b1(2*128) b2(49) msk(128*2) idn(128*128)
    OF = {}
    off = 0
    for nm, sz in (("w7", 3 * 128 * 49), ("b7", 3 * 128), ("b1", 2 * 128),
                   ("b2", 49), ("msk", 128 * 2), ("idn", 128 * 128)):
        OF[nm] = off
        off += sz
    pkf = nc.declare_dram_parameter("pkf", [off], f32, isOutput=False)
    out = nc.declare_dram_parameter("out", [49, 32 * 64], bf16, isOutput=True)

    def pslice(pk, a, p, x):
        return pk[a: a + p * x].rearrange("(p x) -> p x", p=p)
    gdump = nc.dram_tensor("gdump", [NPAIR, 128, 1440], f32)

    with tile.TileContext(nc) as tc:
        with tc.tile_pool(name="big", bufs=1) as big, \
             tc.tile_pool(name="sml", bufs=2) as sml, \
             tc.tile_pool(name="gps", bufs=2, space="PSUM") as gps, \
             tc.tile_pool(name="tps", bufs=1, space="PSUM") as tps, \
             tc.tile_pool(name="aps", bufs=1, space="PSUM") as aps:

            tf18 = big.tile([128, 2, F1R * 64], fp8, tag="tf18")
            for k in range(2):
                nc.sync.dma_start(
                    tf18[:, k], pslice(pk8, k * 128 * F1R * 64, 128, F1R * 64))
            tf28 = big.tile([128, 2, F2R * 80], fp8, tag="tf28")
            nc.vector.memset(tf28[:], 0.0)
            for k in range(2):
                src = pslice(pk8, N1 + k * 128 * F2R * 64, 128, F2R * 64)
                dst = dataclasses.replace(
                    tf28[:, k], offset=tf28[:, k].offset + 8,
                    ap=[[2 * F2R * 80, 128], [80, F2R], [1, 64]])
                nc.sync.dma_start(
                    dst, dataclasses.replace(
                        src, ap=[[F2R * 64, 128], [64, F2R], [1, 64]]))
            tf1 = big.tile([128, 2, F1R * 64], bf16, tag="tf1")
            nc.vector.tensor_copy(tf1[:], tf18[:])
            tf2 = big.tile([128, 2, F2R * 80], bf16, tag="tf2")
            nc.vector.tensor_copy(tf2[:], tf28[:])
            tw7 = big.tile([128, 3, 49], f32, tag="tw7")
            tb7 = big.tile([128, 3, 1], f32, tag="tb7")
            for k in range(3):
                nc.sync.dma_start(
                    tw7[:, k], pslice(pkf, OF["w7"] + k * 128 * 49, 128, 49))
                nc.sync.dma_start(
                    tb7[:, k], pslice(pkf, OF["b7"] + k * 128, 128, 1))
            tw18 = big.tile([128, 3, 9 * 144], fp8, tag="tw18")
            for k in range(3):
                nc.sync.dma_start(
                    tw18[:, k],
                    pslice(pk8, N1 + N2 + k * 128 * 1296, 128, 1296))
            tw1 = big.tile([128, 3, 9 * 144], bf16, tag="tw1")
            nc.vector.tensor_copy(tw1[:], tw18[:])
            tb1 = big.tile([128, 2, 1], f32, tag="tb1")
            for k in range(2):
                nc.sync.dma_start(
                    tb1[:, k], pslice(pkf, OF["b1"] + k * 128, 128, 1))
            tw2 = big.tile([128, 2, 9 * 49], bf16, tag="tw2")
            for k in range(2):
                nc.sync.dma_start(
                    tw2[:, k], pslice(pkw, k * 128 * 441, 128, 441))
            tb2 = big.tile([49, 1], f32, tag="tb2")
            nc.sync.dma_start(tb2[:], pslice(pkf, OF["b2"], 49, 1))
            tmsk = big.tile([128, 2], f32, tag="tmsk")
            nc.sync.dma_start(tmsk[:], pslice(pkf, OF["msk"], 128, 2))
            ident = big.tile([128, 128], f32, tag="ident")
            nc.sync.dma_start(ident[:], pslice(pkf, OF["idn"], 128, 128))

            # cost-volume Grams -> scratch DRAM
            for j in range(NPAIR):
                ps = gps.tile([128, 3, 512], f32, tag="gram")
                for kb in range(2):
                    for k3 in range(3):
                        nc.tensor.matmul(
                            ps[:, k3, :480],
                            tf1[:, kb, j * 128:(j + 1) * 128],
                            tf2[:, kb, j * 160 + 480 * k3:
                                j * 160 + 480 * k3 + 480],
                            start=(kb == 0), stop=(kb == 1))
                gsb = sml.tile([128, 1440], f32, tag="gsb")
                src = dataclasses.replace(
                    ps[:], ap=[[3 * 512, 128], [512, 3], [1, 480]])
                dst = dataclasses.replace(
                    gsb[:], ap=[[1440, 128], [480, 3], [1, 480]])
                nc.scalar.activation(dst, src, AF.Copy)
                nc.sync.dma_start(gdump[j], gsb[:])

            # sheared gather + PE transpose -> cv tiles [o, 42x70] bf16
            cvt = []
            for i in range(3):
                _cv = big.tile([128, 42 * 70], bf16, tag=f"cv{i}", name=f"cv{i}")
                cvt.append(_cv)
            for i in range(3):
                nc.vector.memset(cvt[i][:], 0.0)
            gap = gdump.ap()
            for j in range(NPAIR):
                S = sml.tile([128, 289], f32, tag="S")
                for r in range(2):
                    sap = dataclasses.replace(
                        gap, offset=j * 128 * 1440 + r * 92240,
                        ap=[[1441, 64], [80, 17], [1, 17]])
                    dap = dataclasses.replace(
                        S[:], offset=S[:].offset + r * 289,
                        ap=[[2 * 289, 64], [17, 17], [1, 17]])
                    nc.sync.dma_start(dap, sap)
                for i in range(3):
                    n = OCH[i]
                    tp = tps.tile([128, 128], f32, tag="tp")
                    nc.tensor.transpose(
                        tp[:n, :], S[:, i * 128: i * 128 + n], ident[:])
                    dst = dataclasses.replace(
                        cvt[i][:], offset=cvt[i][:].offset + (2 * j) * 70 + 3,
                        ap=[[42 * 70, n], [70, 2], [1, 64]])
                    src = dataclasses.replace(
                        tp[:], ap=[[128, n], [1, 2], [2, 64]])
                    nc.scalar.activation(dst, src, AF.Lrelu, alpha=0.1)

            # depthwise 7x7 att (f32 acc) + attvol
            avt = []
            for i in range(3):
                _av = big.tile([128, 36 * 66], bf16, tag=f"av{i}", name=f"av{i}")
                avt.append(_av)
            for i in range(3):
                nc.vector.memset(avt[i][:], 0.0)
            for i in range(3):
                n = OCH[i]
                acc = sml.tile([128, 36 * 64], f32, tag="acc")
                tmp = sml.tile([128, 36 * 64], f32, tag="tmp")
                first = True
                for kj in range(7):
                    for ki in range(7):
                        t = kj * 7 + ki
                        src = dataclasses.replace(
                            cvt[i][:n], offset=cvt[i][:].offset + kj * 70 + ki,
                            ap=[[42 * 70, n], [70, 36], [1, 64]])
                        if first:
                            nc.vector.tensor_scalar(
                                acc[:n], src, tw7[:n, i, t:t + 1], tb7[:n, i],
                                op0=mybir.AluOpType.mult,
                                op1=mybir.AluOpType.add)
                            first = False
                        else:
                            nc.vector.tensor_scalar_mul(
                                tmp[:n], src, tw7[:n, i, t:t + 1])
                            nc.vector.tensor_tensor(
                                out=acc[:n], in0=acc[:n], in1=tmp[:n],
                                op=mybir.AluOpType.add)
                cvc = dataclasses.replace(
                    cvt[i][:n], offset=cvt[i][:].offset + 3 * 70 + 3,
                    ap=[[42 * 70, n], [70, 36], [1, 64]])
                avd = dataclasses.replace(
                    avt[i][:n], offset=avt[i][:].offset + 1,
                    ap=[[36 * 66, n], [66, 36], [1, 64]])
                nc.vector.tensor_tensor(
                    out=avd, in0=cvc, in1=acc[:n], op=mybir.AluOpType.mult)

            # agg1 289->144 3x3 (+relu/bias), rows B-1..B+32
            h1a = big.tile([128, 34 * 66], bf16, tag="h1a")
            h1b = big.tile([16, 34 * 66], bf16, tag="h1b")
            h1 = [h1a, h1b]
            for t_ in h1:
                nc.vector.memset(t_[:], 0.0)
            rowblk1 = ((0, 8), (8, 8), (16, 8), (24, 8), (32, 2))
            for ct in range(2):
                co0, con = (0, 128) if ct == 0 else (128, 16)
                for (q0, qn) in rowblk1:
                    ps = aps.tile([128, 512], f32, tag="agg")
                    mm = 0
                    for kb in range(3):
                        n = OCH[kb]
                        for kj in range(3):
                            for ki in range(3):
                                rhs = dataclasses.replace(
                                    avt[kb][:n],
                                    offset=avt[kb][:].offset
                                    + (q0 + kj) * 66 + ki,
                                    ap=[[36 * 66, n], [66, qn], [1, 64]])
                                lhsT = tw1[:n, kb,
                                           (kj * 3 + ki) * 144 + co0:
                                           (kj * 3 + ki) * 144 + co0 + con]
                                nc.tensor.matmul(
                                    ps[:con, :qn * 64], lhsT, rhs,
                                    start=(mm == 0), stop=(mm == 26))
                                mm += 1
                    dst = dataclasses.replace(
                        h1[ct][:con], offset=h1[ct][:].offset + q0 * 66 + 1,
                        ap=[[34 * 66, con], [66, qn], [1, 64]])
                    src = dataclasses.replace(
                        ps[:con], ap=[[512, con], [64, qn], [1, 64]])
                    nc.scalar.activation(dst, src, AF.Relu,
                                         bias=tb1[:con, ct], scale=1.0)
            for ct in range(2):
                con = 128 if ct == 0 else 16
                nc.vector.tensor_scalar_mul(
                    h1[ct][:con, 1:65], h1[ct][:con, 1:65], tmsk[:con, 0:1])
                lo = 33 * 66 + 1
                nc.vector.tensor_scalar_mul(
                    h1[ct][:con, lo:lo + 64], h1[ct][:con, lo:lo + 64],
                    tmsk[:con, 1:2])

            # agg2 144->49 3x3 (+relu/bias), out rows B..B+31
            osb = big.tile([49, 32 * 64], bf16, tag="osb")
            rowblk2 = ((0, 8), (8, 8), (16, 8), (24, 8))
            for (q0, qn) in rowblk2:
                ps = aps.tile([128, 512], f32, tag="agg")
                mm = 0
                for kb in range(2):
                    kn = 128 if kb == 0 else 16
                    for kj in range(3):
                        for ki in range(3):
                            rhs = dataclasses.replace(
                                h1[kb][:kn],
                                offset=h1[kb][:].offset + (q0 + kj) * 66 + ki,
                                ap=[[34 * 66, kn], [66, qn], [1, 64]])
                            lhsT = tw2[:kn, kb, (kj * 3 + ki) * 49:
                                       (kj * 3 + ki) * 49 + 49]
                            nc.tensor.matmul(
                                ps[:49, :qn * 64], lhsT, rhs,
                                start=(mm == 0), stop=(mm == 17))
                            mm += 1
                nc.scalar.activation(
                    osb[:, q0 * 64:(q0 + qn) * 64], ps[:49, :qn * 64],
                    AF.Relu, bias=tb2[:], scale=1.0)
            nc.sync.dma_start(out[:], osb[:])

    _legalize_waits(nc)
    return nc


# ---------------------------------------------------------------------------
# Persistent jitted executable (built once at import).
_RT = {}


def _build_runner():
    import jax
    from jax.sharding import Mesh, PartitionSpec
    from jax.experimental.shard_map import shard_map
    from concourse import bass2jax
    from concourse.bass2jax import _bass_exec_p, partition_id_tensor

    nc = _build_nc()
    bass2jax.install_neuronx_cc_hook()

    in_names, out_names, out_avals, zero_outs = [], [], [], []
    partition_name = (nc.partition_id_tensor.name
                      if nc.partition_id_tensor else None)
    for alloc in nc.m.functions[0].allocations:
        if not isinstance(alloc, mybir.MemoryLocationSet):
            continue
        name = alloc.memorylocations[0].name
        if alloc.kind == "ExternalInput":
            if name != partition_name:
                in_names.append(name)
        elif alloc.kind == "ExternalOutput":
            out_names.append(name)
            shape = tuple(alloc.tensor_shape)
            dtype = mybir.dt.np(alloc.dtype)
            out_avals.append(jax.core.ShapedArray(shape, dtype))
            zero_outs.append(np.zeros(shape, dtype))
    n_params = len(in_names)
    all_names = in_names + out_names
    if partition_name is not None:
        all_names.append(partition_name)
    donate = tuple(range(n_params, n_params + len(out_names)))

    def _body(*args):
        operands = list(args)
        if partition_name is not None:
            operands.append(partition_id_tensor())
        outs = _bass_exec_p.bind(
            *operands, out_avals=tuple(out_avals), in_names=tuple(all_names),
            out_names=tuple(out_names), lowering_input_output_aliases=(),
            sim_require_finite=True, sim_require_nnan=True, nc=nc)
        return tuple(outs)

    devices = jax.devices()[:8]
    mesh = Mesh(np.asarray(devices), ("core",))
    nio = n_params + len(out_names)
    sharded = jax.jit(
        shard_map(_body, mesh=mesh,
                  in_specs=(PartitionSpec("core"),) * nio,
                  out_specs=(PartitionSpec("core"),) * len(out_names),
                  check_rep=False),
        donate_argnums=donate, keep_unused=True)
    _RT.update(sharded=sharded, in_names=in_names, out_names=out_names,
               out_avals=out_avals, zero_outs=zero_outs)


def _run(in_maps):
    sharded = _RT["sharded"]
    in_names, zero_outs = _RT["in_names"], _RT["zero_outs"]
    concat_in = [np.concatenate([m[nm] for m in in_maps], axis=0)
                 for nm in in_names]
    concat_zeros = [np.zeros((8 * z.shape[0], *z.shape[1:]), z.dtype)
                    for z in zero_outs]
    out_arrs = sharded(*concat_in, *concat_zeros)
    av = _RT["out_avals"][0]
    return np.asarray(out_arrs[0]).reshape(8, *av.shape)


# ---------------------------------------------------------------------------
# Host-side input prep / output assembly.

def _prep_static(att_w, att_b, agg1_w, agg1_b, agg2_w, agg2_b):
    bf = ml_dtypes.bfloat16
    w7 = np.zeros((3, 128, 49), np.float32)
    b7 = np.zeros((3, 128, 1), np.float32)
    aw = att_w.reshape(289, 49)
    for i in range(3):
        n = min(128, 289 - 128 * i)
        w7[i, :n] = aw[128 * i:128 * i + n]
        b7[i, :n, 0] = att_b[128 * i:128 * i + n]
    w1 = np.zeros((3, 128, 9, 144), np.float32)
    for kb in range(3):
        n = min(128, 289 - 128 * kb)
        w1[kb, :n] = agg1_w[:, 128 * kb:128 * kb + n].reshape(144, n, 9)\
            .transpose(1, 2, 0)
    b1 = np.zeros((2, 128, 1), np.float32)
    b1[0, :, 0] = agg1_b[:128]
    b1[1, :16, 0] = agg1_b[128:]
    w2 = np.zeros((2, 128, 9, 49), np.float32)
    w2[0] = agg2_w[:, :128].reshape(49, 128, 9).transpose(1, 2, 0)
    w2[1, :16] = agg2_w[:, 128:].reshape(49, 16, 9).transpose(1, 2, 0)
    w1f8 = np.ascontiguousarray(w1).astype(ml_dtypes.float8_e4m3).ravel()
    pkw = np.ascontiguousarray(w2).astype(ml_dtypes.bfloat16).ravel()
    pkf_base = np.concatenate([
        w7.ravel(), b7.ravel(), b1.ravel(),
        agg2_b.astype(np.float32).ravel()])
    return pkw, pkf_base, w1f8


def _prep_inputs(f1, f2, att_w, att_b, agg1_w, agg1_b, agg2_w, agg2_b):
    bf = ml_dtypes.float8_e4m3
    pkw, pkf_base, w1f8 = _prep_static(
        att_w, att_b, agg1_w, agg1_b, agg2_w, agg2_b)
    idn = np.eye(128, dtype=np.float32).ravel()
    nrm = np.sqrt(np.einsum('bchw,bchw->bhw', f1, f1))
    alpha = 1.0 / (np.maximum(nrm, 1e-12) * C)
    f1n = (f1 * alpha[:, None]).astype(bf)
    f2q = f2.astype(bf)
    in_maps = []
    for core in range(8):
        b, half = core // 2, core % 2
        Bb = half * 32
        f1s = np.zeros((C, F1R, 64), bf)
        lo, hi = Bb - 5, Bb + 37
        clo, chi = max(lo, 0), min(hi, H)
        f1s[:, clo - lo:chi - lo] = f1n[b, :, clo:chi]
        f2s = np.zeros((C, F2R, 64), bf)
        lo2, hi2 = Bb - 13, Bb + 45
        clo2, chi2 = max(lo2, 0), min(hi2, H)
        f2s[:, clo2 - lo2:chi2 - lo2] = f2q[b, :, clo2:chi2]
        msk = np.zeros((128, 2), np.float32)
        msk[:, 0] = 0.0 if half == 0 else 1.0
        msk[:, 1] = 1.0 if half == 0 else 0.0
        pk8 = np.concatenate([f1s.ravel(), f2s.ravel(), w1f8])
        pkf = np.concatenate([pkf_base, msk.ravel(), idn])
        in_maps.append({"pk8": pk8, "pkw": pkw, "pkf": pkf})
    return in_maps


def kernel(f1, f2, att_w, att_b, agg1_w, agg1_b, agg2_w, agg2_b):
    f1 = np.asarray(f1, np.float32)
    f2 = np.asarray(f2, np.float32)
    in_maps = _prep_inputs(
        f1, f2,
        np.asarray(att_w, np.float32), np.asarray(att_b, np.float32),
        np.asarray(agg1_w, np.float32), np.asarray(agg1_b, np.float32),
        np.asarray(agg2_w, np.float32), np.asarray(agg2_b, np.float32))
    res = _run(in_maps)   # [8, 49, 2048] bf16
    out = np.empty((B_, 49, H, W), np.float32)
    for core in range(8):
        b, half = core // 2, core % 2
        out[b, :, half * 32:half * 32 + 32] = \
            res[core].astype(np.float32).reshape(49, 32, 64)
    return out


# ---- compile + warm up at import ------------------------------------------
_build_runner()
_warm = _prep_inputs(
    np.ones((B_, C, H, W), np.float32), np.ones((B_, C, H, W), np.float32),
    np.zeros((289, 1, 7, 7), np.float32), np.zeros(289, np.float32),
    np.zeros((144, 289, 3, 3), np.float32), np.zeros(144, np.float32),
    np.zeros((49, 144, 3, 3), np.float32), np.zeros(49, np.float32))
_run(_warm)
del _warm
